# revision 1
# baseline (speedup 1.0000x reference)
"""DiffAttention Trainium2 kernel (8-core SPMD, full-I/O contract).

Sharding: core c = (batch b = c//4) x (head-group g = c%4, 4 of 16 v-heads).
Each core computes qkv for its heads, two sub-attentions (differential),
softmax via exp + ones-column denominator trick, combine + RMSNorm, and a
partial output projection o @ W_proj[rows].  Host sums the 4 partials per
batch and adds b_proj.

Device layout notes:
  - x is uploaded pre-transposed (xT [1024, 2048]) so both qkv matmul
    orientations need no on-device transposes.
  - scores are computed transposed (S^T [ktok, qtok]) so the AV matmul can
    keep v stationary and stream expS^T; the softmax denominator comes from
    an appended ones column in v (row 64 of the AV accumulator).
  - all matmuls run as float32r (full-rate fp32 PE mode; every moving dim is
    >= 256 so it streams at 1 cycle/row).
  - RMSNorm rstd is computed as exp(-0.5*ln(ms+eps)) so only one ACT table
    set (natural_log_exp) is ever needed; the cross-partition sums for the
    mean-square and the normalizer broadcasts run on GPSIMD
    (partition_all_reduce / partition_broadcast), keeping PSUM free.
  - PSUM: 2-bank score tiles double-buffered (4 banks) + one 4-bank slot
    rotating O1 -> O2 -> proj; the projection of q-block N is emitted inside
    q-block N+1's attention so it never stalls the exp pipeline.
  - W_qkv q-columns are pre-scaled by hd^-0.5, W_proj rows by
    subln_w * (1 - lambda_init) on the host.
  - x/W live in a phase-1 pool that is released before the attention-phase
    buffers are allocated, so peak SBUF stays under budget.
"""

import numpy as np

P = 128
N_TOK = 2048
DIM = 1024
NCORES = 8
HD = 32
VD = 64            # 2*hd, v-head dim
VD1 = VD + 1       # + ones column for the softmax denominator
NKD = DIM // P     # 8 k-tiles over the model dim
NKT = N_TOK // P   # 16 token tiles
QB = 512           # query block
NQB = N_TOK // QB  # 4
LAMBDA_INIT = 0.8 - 0.6 * float(np.exp(-0.3 * 12))
EPS = 1e-5
SCALE = HD ** -0.5

_CACHE: dict = {}


def _build_module():
    from contextlib import ExitStack

    import concourse.bass as bass  # noqa: F401
    import concourse.mybir as mybir
    import concourse.tile as tile
    from concourse import bacc, bass_isa

    f32 = mybir.dt.float32
    f32r = mybir.dt.float32r
    AF = mybir.ActivationFunctionType

    nc = bacc.Bacc(
        "TRN2", target_bir_lowering=False, debug=False, num_devices=NCORES
    )

    xT_d = nc.dram_tensor("xt", [DIM, N_TOK], f32r, kind="ExternalInput").ap()
    wqkv_d = nc.dram_tensor("wqkv", [DIM, 768], f32r, kind="ExternalInput").ap()
    wproj_d = nc.dram_tensor("wproj", [4 * VD, DIM], f32r, kind="ExternalInput").ap()
    vones_d = nc.dram_tensor("vones", [P, NKT * 4], f32r, kind="ExternalInput").ap()
    neglam_d = nc.dram_tensor("neglam", [1, 1], f32, kind="ExternalInput").ap()
    out_d = nc.dram_tensor("outp", [N_TOK, DIM], f32, kind="ExternalOutput").ap()
    dbg = {}
    if _CACHE.get("debug"):
        dbg["qk"] = nc.dram_tensor("dbg_qk", [P, 4 * QB], f32, kind="ExternalOutput").ap()
        dbg["vx"] = nc.dram_tensor("dbg_vx", [P, 4 * VD1], f32, kind="ExternalOutput").ap()
        dbg["et"] = nc.dram_tensor("dbg_et", [P, 2 * QB], f32, kind="ExternalOutput").ap()
        dbg["o1"] = nc.dram_tensor("dbg_o1", [VD1, 4 * QB], f32, kind="ExternalOutput").ap()
        dbg["ot"] = nc.dram_tensor("dbg_ot", [VD, 4 * QB], f32, kind="ExternalOutput").ap()
        dbg["r1"] = nc.dram_tensor("dbg_r1", [VD, 4 * QB], f32, kind="ExternalOutput").ap()
        dbg["rstd"] = nc.dram_tensor("dbg_rstd", [VD, 4 * QB], f32, kind="ExternalOutput").ap()

    with ExitStack() as ctx:
        tc = ctx.enter_context(tile.TileContext(nc))

        singles = ctx.enter_context(tc.tile_pool(name="singles", bufs=1))
        ps_s = ctx.enter_context(tc.tile_pool(name="ps_s", bufs=2, space="PSUM"))
        ps_o = ctx.enter_context(tc.tile_pool(name="ps_o", bufs=1, space="PSUM"))

        # qk tiles: [m][n] -> (x @ Wm)^T chunk, m in (q1, q2, k1, k2), n = tok/512
        qk_sb = [
            [singles.tile([P, QB], f32r, tag=f"qk{m}_{n}", name=f"qk{m}_{n}") for n in range(NQB)]
            for m in range(4)
        ]
        # v tiles per token-tile, with the denominator ones column appended
        vx_sb = [singles.tile([P, 4, VD1], f32r, tag=f"vx{t}", name=f"vx{t}") for t in range(NKT)]
        wp_sb = singles.tile([VD, 4, DIM], f32r, tag="wp")
        ones64 = singles.tile([VD, VD], f32r, tag="ones64")
        neglam_sb = singles.tile([1, 1], f32, tag="nl")
        eps_sb = singles.tile([VD, 1], f32, tag="eps")

        nc.vector.memset(eps_sb, EPS)

        # ---- stage 1: qkv projections (x/W pool released afterwards) ----
        # qk_sb[:, m, :] = (x @ Wm)^T for m in (q1, q2, k1, k2); v in token-major
        # layout with a ones column appended per head.
        with tc.tile_pool(name="ph1", bufs=1) as ph1:
            # per-k-tile loads so qkv matmuls can chase the x DMAs
            xT_t = xT_d.rearrange("(ko p) t -> ko p t", p=P)
            wq_t = wqkv_d.rearrange("(ko p) c -> ko p c", p=P)
            x_sb = []
            w_sb = []
            for k in range(NKD):
                wt = ph1.tile([P, 768], f32r, tag=f"w{k}", name=f"w{k}")
                nc.sync.dma_start(wt, wq_t[k])
                w_sb.append(wt)
                xt = ph1.tile([P, N_TOK], f32r, tag=f"x{k}", name=f"x{k}")
                nc.sync.dma_start(xt, xT_t[k])
                x_sb.append(xt)
            # small late loads - not needed until AV / projection
            for t in range(NKT):
                nc.sync.dma_start(
                    vx_sb[t][:, :, VD:VD1],
                    vones_d.rearrange("p (t j) -> p t j", j=4)[:, t, :].unsqueeze(2),
                )
            nc.sync.dma_start(wp_sb, wproj_d.rearrange("(j v) c -> v j c", v=VD))
            nc.sync.dma_start(ones64, vones_d[0:VD, 0:VD])
            nc.sync.dma_start(neglam_sb, neglam_d)

            def qk_group(m, n):
                ps = ps_s.tile([P, 2 * QB], f32, tag="s", name="s1qk")
                pqk = ps[:, :QB]
                for k in range(NKD):
                    nc.tensor.matmul(
                        pqk,
                        lhsT=w_sb[k][:, m * P:(m + 1) * P],
                        rhs=x_sb[k][:, n * QB:(n + 1) * QB],
                        start=(k == 0),
                        stop=(k == NKD - 1),
                    )
                nc.vector.tensor_copy(qk_sb[m][n], pqk)

            def v_group(i):
                po = ps_o.tile([P, 4 * QB], f32, tag="o", name="s1v")
                pv = po[:, :4 * VD]
                for k in range(NKD):
                    nc.tensor.matmul(
                        pv,
                        lhsT=x_sb[k][:, i * P:(i + 1) * P],
                        rhs=w_sb[k][:, 512:768],
                        start=(k == 0),
                        stop=(k == NKD - 1),
                    )
                nc.vector.tensor_copy(
                    vx_sb[i][:, :, 0:VD], pv.rearrange("p (j v) -> p j v", j=4)
                )

            # v first (its psum pool slot must be free before the first
            # AV accumulator is allocated), then q/k in consumption order.
            for i in range(NKT):
                v_group(i)
            for n in range(NQB):
                for m in (0, 2, 1, 3):
                    qk_group(m, n)

        if dbg:
            dq = singles.tile([P, 4 * QB], f32, tag="dbgq")
            for m in range(4):
                nc.vector.tensor_copy(dq[:, m * QB:(m + 1) * QB], qk_sb[m][0])
            nc.sync.dma_start(dbg["qk"], dq)
            dv = singles.tile([P, 4 * VD1], f32, tag="dbgv")
            nc.vector.tensor_copy(dv.rearrange("p (j v) -> p j v", j=4), vx_sb[0])
            nc.sync.dma_start(dbg["vx"], dv)

        # ---- stage 2+3 pools (reuse the released x/W space) ----
        expp = ctx.enter_context(tc.tile_pool(name="expp", bufs=14))
        bcast = ctx.enter_context(tc.tile_pool(name="bcast", bufs=4))
        stage = ctx.enter_context(tc.tile_pool(name="stage", bufs=4))
        ocp = ctx.enter_context(tc.tile_pool(name="ocp", bufs=2))
        owk = ctx.enter_context(tc.tile_pool(name="owk", bufs=2))
        owk1 = ctx.enter_context(tc.tile_pool(name="owk1", bufs=1))

        def emit_proj_half(o_t, qb, half):
            """Partial projection for tok tiles (2*half, 2*half+1) of q-block qb."""
            q0 = qb * QB
            pp = ps_o.tile([P, 4 * QB], f32, tag="o")
            for sl in range(4):
                t = half * 2 + sl // 2
                nck = sl % 2
                outsl = pp[:, sl * QB:(sl + 1) * QB]
                for j in range(4):
                    nc.tensor.matmul(
                        outsl,
                        lhsT=o_t[:, j * QB + t * P:j * QB + (t + 1) * P],
                        rhs=wp_sb[:, j, nck * QB:(nck + 1) * QB],
                        start=(j == 0),
                        stop=(j == 3),
                        skip_group_check=True,
                    )
                st = stage.tile([P, QB], f32, tag="st")
                nc.vector.tensor_copy(st, outsl)
                nc.sync.dma_start(
                    out_d[q0 + t * P:q0 + (t + 1) * P, nck * QB:(nck + 1) * QB], st
                )

        def emit_combine_chunk(o1s, o2s, o_t, lo, hi):
            """Normalize both groups, differential combine, RMSNorm for
            columns [lo:hi) (contiguous sub-head blocks). Pure
            SBUF/DVE/Pool/ACT - no PSUM, so it overlaps attention freely."""
            w = hi - lo
            # DVE lanes are per-partition: move the denominator row (partition
            # VD) to partition 0 via DMA before computing reciprocals.
            r1b = bcast.tile([VD, 4 * QB], f32, tag="b", name="r1b")[:, :w]
            nc.sync.dma_start(r1b[0:1, :], o1s[VD:VD1, lo:hi])
            nc.vector.reciprocal_approx_fast(r1b[0:1, :], r1b[0:1, :])
            nc.gpsimd.partition_broadcast(r1b, r1b[0:1, :])
            r2b = bcast.tile([VD, 4 * QB], f32, tag="b", name="r2b")[:, :w]
            nc.sync.dma_start(r2b[0:1, :], o2s[VD:VD1, lo:hi])
            nc.vector.reciprocal_approx_fast(r2b[0:1, :], r2b[0:1, :])
            nc.vector.tensor_scalar_mul(r2b[0:1, :], r2b[0:1, :], neglam_sb[0:1, 0:1])
            nc.gpsimd.partition_broadcast(r2b, r2b[0:1, :])

            # o = attn1 - lam * attn2  (written as f32r for the projection)
            ot = o_t[:, lo:hi]
            sq_t = owk1.tile([VD, 4 * QB], f32, tag="sq", name="sq_t")[:, :w]
            nc.vector.tensor_mul(ot, o1s[0:VD, lo:hi], r1b)
            nc.vector.tensor_mul(r2b, o2s[0:VD, lo:hi], r2b)
            nc.vector.tensor_add(ot, ot, r2b)

            # RMSNorm over vd: ms broadcast to all rows via partition_all_reduce
            nc.vector.tensor_mul(sq_t, ot, ot)
            ssqb = bcast.tile([VD, 4 * QB], f32, tag="b", name="ssqb")[:, :w]
            nc.gpsimd.partition_all_reduce(ssqb, sq_t, VD, bass_isa.ReduceOp.add)
            rstd_t = bcast.tile([VD, 4 * QB], f32, tag="b", name="rstd_t")[:, :w]
            nc.scalar.activation(rstd_t, ssqb, AF.Ln, bias=eps_sb, scale=1.0 / VD)
            rstd = bcast.tile([VD, 4 * QB], f32, tag="b", name="rstd")[:, :w]
            nc.scalar.activation(rstd, rstd_t, AF.Exp, scale=-0.5)
            nc.vector.tensor_mul(ot, ot, rstd)

        def emit_combine(o1s, o2s, nchunks=1):
            o_t = owk.tile([VD, 4 * QB], f32r, tag="o", name="o_t")
            step = 4 * QB // nchunks
            for c in range(nchunks):
                emit_combine_chunk(o1s, o2s, o_t, c * step, (c + 1) * step)
            return o_t

        # ---- attention, with combine and projection of q-block N software-
        # pipelined into q-block N+1's group-1 attention ----
        prevo = None  # (o1s, o2s, qb) awaiting combine
        prev = None   # (o tile, qb) whose projection is still pending
        for qb in range(NQB):
            q0 = qb * QB
            og_sb = []
            for g in range(2):
                po = ps_o.tile([VD1, 4 * QB], f32, tag="o")
                for kt in range(NKT):
                    if g == 0 and kt == 2 and prevo is not None:
                        prev = (emit_combine(prevo[0], prevo[1]), prevo[2])
                        prevo = None
                    for h in range(2):
                        ps = ps_s.tile([P, 2 * QB], f32, tag="s")
                        for jj in range(2):
                            j = 2 * h + jj
                            # S^T[kt-block, q-block] for sub-head j of group g
                            nc.tensor.matmul(
                                ps[:, jj * QB:(jj + 1) * QB],
                                lhsT=qk_sb[2 + g][kt // NQB][
                                    32 * j:32 * (j + 1),
                                    (kt % NQB) * P:(kt % NQB + 1) * P,
                                ],
                                rhs=qk_sb[g][qb][32 * j:32 * (j + 1), :],
                                start=True,
                                stop=True,
                                tile_position=(32 * j, 0),
                            )
                        et = expp.tile([P, 2 * QB], f32r, tag="e")
                        nc.scalar.activation(et, ps, AF.Exp)
                        if dbg and qb == 0 and g == 0 and kt == 0 and h == 0:
                            de = singles.tile([P, 2 * QB], f32, tag="dbge")
                            nc.vector.tensor_copy(de, et)
                            nc.sync.dma_start(dbg["et"], de)
                        for jj in range(2):
                            j = 2 * h + jj
                            nc.tensor.matmul(
                                po[:, j * QB:(j + 1) * QB],
                                lhsT=vx_sb[kt][:, j, :],
                                rhs=et[:, jj * QB:(jj + 1) * QB],
                                start=(kt == 0),
                                stop=(kt == NKT - 1),
                                skip_group_check=True,
                            )
                og = ocp.tile([VD1, 4 * QB], f32, tag="og")
                nc.vector.tensor_copy(og, po)
                og_sb.append(og)
                if qb == NQB - 1 and g == 0:
                    # final block: overlap the group-1 normalizer with group-2
                    o_last = owk.tile([VD, 4 * QB], f32r, tag="o", name="o_last")
                    r1b_l = bcast.tile([VD, 4 * QB], f32, tag="b", name="r1b_l")
                    nc.sync.dma_start(r1b_l[0:1, :], og[VD:VD1, :])
                    nc.vector.reciprocal_approx_fast(r1b_l[0:1, :], r1b_l[0:1, :])
                    nc.gpsimd.partition_broadcast(r1b_l, r1b_l[0:1, :])
                    nc.vector.tensor_mul(o_last, og[0:VD, :], r1b_l)
                if dbg and qb == 0 and g == 0:
                    nc.sync.dma_start(dbg["o1"], og)
                if g == 0 and prev is not None:
                    emit_proj_half(*prev, half=0)
            if prev is not None:
                emit_proj_half(*prev, half=1)
                prev = None

            prevo = (og_sb[0], og_sb[1], qb)

        # tail: group-2 normalizer, differential combine, RMS, projection for
        # the final q-block (its group-1 half was computed during group 2)
        o2s = prevo[1]
        r2b_l = bcast.tile([VD, 4 * QB], f32, tag="b", name="r2b_l")
        nc.sync.dma_start(r2b_l[0:1, :], o2s[VD:VD1, :])
        nc.vector.reciprocal_approx_fast(r2b_l[0:1, :], r2b_l[0:1, :])
        nc.vector.tensor_scalar_mul(r2b_l[0:1, :], r2b_l[0:1, :], neglam_sb[0:1, 0:1])
        nc.gpsimd.partition_broadcast(r2b_l, r2b_l[0:1, :])
        nc.vector.tensor_mul(r2b_l, o2s[0:VD, :], r2b_l)
        nc.vector.tensor_add(o_last, o_last, r2b_l)
        sq_l = owk1.tile([VD, 4 * QB], f32r, tag="sq", name="sq_l")
        nc.vector.tensor_mul(sq_l, o_last, o_last)
        # RMS partition-reduce on the (idle) PE via ones-matmul, in the free
        # double-buffered score-psum slots; ln/exp per 2-chunk
        rstd_tl = bcast.tile([VD, 4 * QB], f32, tag="b", name="rstd_tl")
        rstd_l = bcast.tile([VD, 4 * QB], f32, tag="b", name="rstd_l")
        for c in range(2):
            pq = ps_s.tile([P, 2 * QB], f32, tag="s", name=f"ssqp{c}")
            for cc in range(2):
                nc.tensor.matmul(
                    pq[0:VD, cc * QB:(cc + 1) * QB],
                    lhsT=ones64,
                    rhs=sq_l[:, (2 * c + cc) * QB:(2 * c + cc + 1) * QB],
                    start=True,
                    stop=True,
                )
            sl = slice(2 * c * QB, (2 * c + 2) * QB)
            nc.scalar.activation(
                rstd_tl[:, sl], pq[0:VD, :], AF.Ln, bias=eps_sb, scale=1.0 / VD
            )
            nc.scalar.activation(rstd_l[:, sl], rstd_tl[:, sl], AF.Exp, scale=-0.5)
        nc.vector.tensor_mul(o_last, o_last, rstd_l)
        # final projection as 4 pipelined quarters on the score-psum slots
        for t in range(4):
            pq = ps_s.tile([P, 2 * QB], f32, tag="s", name=f"projq{t}")
            for nck in range(2):
                outsl = pq[:, nck * QB:(nck + 1) * QB]
                for j in range(4):
                    nc.tensor.matmul(
                        outsl,
                        lhsT=o_last[:, j * QB + t * P:j * QB + (t + 1) * P],
                        rhs=wp_sb[:, j, nck * QB:(nck + 1) * QB],
                        start=(j == 0),
                        stop=(j == 3),
                        skip_group_check=True,
                    )
                st = stage.tile([P, QB], f32, tag="st", name=f"stq{t}_{nck}")
                nc.vector.tensor_copy(st, outsl)
                nc.sync.dma_start(
                    out_d[
                        prevo[2] * QB + t * P:prevo[2] * QB + (t + 1) * P,
                        nck * QB:(nck + 1) * QB,
                    ],
                    st,
                )

    nc.compile()
    return nc


def _get_module():
    if "nc" not in _CACHE:
        _CACHE["nc"] = _build_module()
    return _CACHE["nc"]


def make_in_maps(inputs: dict) -> list:
    x = np.asarray(inputs["x"], np.float32)
    wqkv = np.asarray(inputs["W_qkv"], np.float32)
    wproj = np.asarray(inputs["W_proj"], np.float32)
    lq1 = np.asarray(inputs["lambda_q1"], np.float32)
    lk1 = np.asarray(inputs["lambda_k1"], np.float32)
    lq2 = np.asarray(inputs["lambda_q2"], np.float32)
    lk2 = np.asarray(inputs["lambda_k2"], np.float32)
    subw = np.asarray(inputs["subln_w"], np.float32)

    lam = float(
        np.exp(np.sum(lq1 * lk1)) - np.exp(np.sum(lq2 * lk2)) + LAMBDA_INIT
    )
    neglam = np.array([[-lam]], np.float32)
    vones = np.ones((P, NKT * 4), np.float32)
    wp_rowscale = (np.tile(subw, 4) * (1.0 - LAMBDA_INIT)).astype(np.float32)

    in_maps = []
    for c in range(NCORES):
        b, g = divmod(c, 4)
        xT = np.ascontiguousarray(x[b].T).astype(np.float32)
        ws = np.ascontiguousarray(
            np.concatenate(
                [
                    wqkv[:, 128 * g:128 * g + 128] * SCALE,
                    wqkv[:, 512 + 128 * g:512 + 128 * g + 128] * SCALE,
                    wqkv[:, 1024 + 128 * g:1024 + 128 * g + 128],
                    wqkv[:, 1536 + 128 * g:1536 + 128 * g + 128],
                    wqkv[:, 2048 + 256 * g:2048 + 256 * g + 256],
                ],
                axis=1,
            )
        ).astype(np.float32)
        wp = np.ascontiguousarray(
            wproj[256 * g:256 * (g + 1), :] * wp_rowscale[:, None]
        ).astype(np.float32)
        in_maps.append(
            {"xt": xT, "wqkv": ws, "wproj": wp, "neglam": neglam, "vones": vones}
        )
    return in_maps


def combine_outputs(inputs: dict, parts: list) -> np.ndarray:
    bproj = np.asarray(inputs["b_proj"], np.float32)
    out = np.stack(
        [
            parts[0] + parts[1] + parts[2] + parts[3],
            parts[4] + parts[5] + parts[6] + parts[7],
        ]
    )
    return (out + bproj[None, None, :]).astype(np.float32)


def kernel(**inputs) -> np.ndarray:
    from concourse import bass_utils

    nc = _get_module()
    in_maps = make_in_maps(inputs)
    res = bass_utils.run_bass_kernel_spmd(nc, in_maps, core_ids=list(range(NCORES)))
    parts = [np.asarray(res.results[c]["outp"], np.float32) for c in range(NCORES)]
    return combine_outputs(inputs, parts)



# revision 4
# speedup vs baseline: 1.0115x; 1.0115x over previous
"""DiffAttention Trainium2 kernel (8-core SPMD, full-I/O contract), v2.

Sharding: core c = (batch b = c//4) x (head-group g = c%4, 4 of 16 v-heads).

Key design points (cost-model-driven rewrite of the v1 baseline):
  - All PE inputs are fp16 (1 cycle/row at ANY moving size, vs f32r which
    needs >=256).  End-to-end rel err ~6e-4 (measured in numpy), budget 2e-2.
  - Scores are computed transposed (S^T [ktok 128, qtok 512]) as before, but
    the AV matmul is flipped to out[q, vd]: lhsT = expS^T chunk [128k, 128q],
    rhs = v-head [128k, 66] -> ap=66 per matmul instead of 512.  This cuts
    the AV stream time ~7.8x (cost = moving dim only).
  - v tiles carry TWO extra columns: ones (group-1 softmax denominator) and
    -1/lambda (group-2), so both AV accumulators come out of PSUM with their
    reciprocal-ready denominators at cols 64/65 of each 66-block.
  - Softmax normalization, differential combine and RMSNorm all happen in
    the natural [q, vd] orientation: per-partition-scalar ops on Pool/DVE,
    no partition broadcasts, no GPSIMD reductions.
  - o_n is transposed back with two PE transpose ops per q-tile (ap=128) and
    projected with contract=128 (2 passes instead of 4): proj drops 2x.
  - The exp softmax (the ACT bottleneck, 256 x [128,1024] tiles) can be
    split by q-columns between ACT (table exp) and DVE (dual-Schraudolph
    bit-trick exp, 3 ops/tile-slice, rel rms ~0.5%).  Rows stay pure so the
    systematic part cancels in the softmax normalization.  SCHR_Q controls
    the number of q-columns (per 512-block) done on DVE.
  - combine/proj of q-block N are software-pipelined into q-block N+1's
    group-0 attention stream (emission order = engine order).
"""

import numpy as np

P = 128
N_TOK = 2048
DIM = 1024
NCORES = 8
HD = 32
VD = 64            # v-head dim
VD2 = VD + 2       # + ones column (g0 denom) + (-1/lam) column (g1 denom)
NKD = DIM // P     # 8 k-chunks over the model dim
NKT = N_TOK // P   # 16 token tiles
QB = 512           # query block
NQB = N_TOK // QB  # 4
NQT = QB // P      # 4 q-tiles per block
LAMBDA_INIT = 0.8 - 0.6 * float(np.exp(-0.3 * 12))
EPS = 1e-5
SCALE = HD ** -0.5

# exp split: number of q-columns (of each 512-q block) computed on DVE via
# dual-Schraudolph.  0 = everything on ACT.
SCHR_Q = 0
# dual-Schraudolph constants (fp16 bit trick, see docstring)
SCHR_K = float(1.4426950408889634 * 1024.0)
SCHR_C = 0.03
SCHR_B1 = float(15360.0 + 0.5 - SCHR_C * 1024.0 - 1024.0)
SCHR_B2 = float(15360.0 + 0.5 - SCHR_C * 1024.0 - 512.0)
SCHR_W2 = float(np.sqrt(2.0) / 2.0)

_CACHE: dict = {}


def _build_module():
    from contextlib import ExitStack

    import concourse.bass as bass  # noqa: F401
    import concourse.mybir as mybir
    import concourse.tile as tile
    from concourse import bacc

    f32 = mybir.dt.float32
    f16 = mybir.dt.float16
    i16 = mybir.dt.int16
    AF = mybir.ActivationFunctionType
    MUL = mybir.AluOpType.mult
    ADD = mybir.AluOpType.add

    nc = bacc.Bacc(
        "TRN2", target_bir_lowering=False, debug=False, num_devices=NCORES
    )

    xT_d = nc.dram_tensor("xt", [DIM, N_TOK], f16, kind="ExternalInput").ap()
    wqkv_d = nc.dram_tensor("wqkv", [DIM, 768], f16, kind="ExternalInput").ap()
    wproj_d = nc.dram_tensor("wproj", [P, 2 * DIM], f16, kind="ExternalInput").ap()
    vcols_d = nc.dram_tensor("vcols", [P, NKT * 8], f16, kind="ExternalInput").ap()
    ident_d = nc.dram_tensor("ident", [P, P], f16, kind="ExternalInput").ap()
    out_d = nc.dram_tensor("outp", [N_TOK, DIM], f32, kind="ExternalOutput").ap()

    with ExitStack() as ctx:
        tc = ctx.enter_context(tile.TileContext(nc))

        singles = ctx.enter_context(tc.tile_pool(name="singles", bufs=1))
        ps_s = ctx.enter_context(tc.tile_pool(name="ps_s", bufs=2, space="PSUM"))
        ps_av = ctx.enter_context(tc.tile_pool(name="ps_av", bufs=4, space="PSUM"))
        etp = ctx.enter_context(tc.tile_pool(name="etp", bufs=3))
        ogp = ctx.enter_context(tc.tile_pool(name="ogp", bufs=8))
        onp = ctx.enter_context(tc.tile_pool(name="onp", bufs=4))
        rp = ctx.enter_context(tc.tile_pool(name="rp", bufs=24))
        sqp = ctx.enter_context(tc.tile_pool(name="sqp", bufs=2))
        otp = ctx.enter_context(tc.tile_pool(name="otp", bufs=4))
        stage = ctx.enter_context(tc.tile_pool(name="stage", bufs=3))

        # qk tiles: [m][n] -> (x @ Wm)^T chunk, m in (q1, q2, k1, k2), n = tok/512
        qk_sb = [
            [singles.tile([P, QB], f16, tag=f"qk{m}_{n}", name=f"qk{m}_{n}") for n in range(NQB)]
            for m in range(4)
        ]
        vx_sb = [singles.tile([P, 4, VD2], f16, tag=f"vx{t}", name=f"vx{t}") for t in range(NKT)]
        wp_sb = singles.tile([P, 2, DIM], f16, tag="wp")
        ident_sb = singles.tile([P, P], f16, tag="id")
        eps_sb = singles.tile([P, 1], f32, tag="eps")

        nc.vector.memset(eps_sb, EPS)
        nc.sync.dma_start(ident_sb, ident_d)
        nc.sync.dma_start(wp_sb, wproj_d.rearrange("p (c n) -> p c n", c=2))

        # ---- stage 1: qkv projections ----
        with tc.tile_pool(name="ph1", bufs=1) as ph1:
            xT_t = xT_d.rearrange("(ko p) t -> ko p t", p=P)
            wq_t = wqkv_d.rearrange("(ko p) c -> ko p c", p=P)
            x_sb = []
            w_sb = []
            for k in range(NKD):
                wt = ph1.tile([P, 768], f16, tag=f"w{k}", name=f"w{k}")
                nc.sync.dma_start(wt, wq_t[k])
                w_sb.append(wt)
                xt = ph1.tile([P, N_TOK], f16, tag=f"x{k}", name=f"x{k}")
                nc.sync.dma_start(xt, xT_t[k])
                x_sb.append(xt)
            for t in range(NKT):
                nc.sync.dma_start(
                    vx_sb[t][:, :, VD:VD2],
                    vcols_d.rearrange("p (t j c) -> p t j c", j=4, c=2)[:, t],
                )

            def qk_group(m, n, eng):
                pqk = ps_av.tile([P, QB], f32, tag="av", name="s1qk")
                for k in range(NKD):
                    nc.tensor.matmul(
                        pqk,
                        lhsT=w_sb[k][:, m * P:(m + 1) * P],
                        rhs=x_sb[k][:, n * QB:(n + 1) * QB],
                        start=(k == 0),
                        stop=(k == NKD - 1),
                    )
                if eng == 0:
                    nc.vector.tensor_copy(qk_sb[m][n], pqk)
                else:
                    nc.scalar.copy(qk_sb[m][n], pqk)

            def v_group(i, eng):
                pv = ps_av.tile([P, QB], f32, tag="av", name="s1v")[:, :4 * VD]
                for k in range(NKD):
                    nc.tensor.matmul(
                        pv,
                        lhsT=x_sb[k][:, i * P:(i + 1) * P],
                        rhs=w_sb[k][:, 512:768],
                        start=(k == 0),
                        stop=(k == NKD - 1),
                    )
                dst = vx_sb[i][:, :, 0:VD]
                src = pv.rearrange("p (j v) -> p j v", j=4)
                if eng == 0:
                    nc.vector.tensor_copy(dst, src)
                else:
                    nc.scalar.copy(dst, src)

            # production in consumption order: q1/k1 for g0 first, v tiles,
            # then q2/k2 (needed only when g1 starts).
            qk_group(0, 0, 0)
            qk_group(2, 0, 1)
            for i in range(4):
                v_group(i, i % 2)
            qk_group(2, 1, 0)
            for i in range(4, 8):
                v_group(i, i % 2)
            qk_group(2, 2, 1)
            for i in range(8, 12):
                v_group(i, i % 2)
            qk_group(2, 3, 0)
            for i in range(12, 16):
                v_group(i, i % 2)
            qk_group(1, 0, 1)
            qk_group(3, 0, 0)
            qk_group(3, 1, 1)
            qk_group(3, 2, 0)
            qk_group(3, 3, 1)
            for n in range(1, NQB):
                qk_group(0, n, n % 2)
                qk_group(1, n, (n + 1) % 2)

        # ---- attention ----

        def emit_exp(et, ps):
            """expS^T for one head-pair tile: ACT exp on the first
            QB-SCHR_Q q-columns of each 512 block, DVE dual-Schraudolph on
            the rest (rows stay implementation-pure across kt)."""
            qs = QB - SCHR_Q
            et3 = et.rearrange("p (h q) -> p h q", h=2)
            ps3 = ps.rearrange("p (h q) -> p h q", h=2)
            if qs > 0:
                nc.scalar.activation(et3[:, :, :qs], ps3[:, :, :qs], AF.Exp)
            if SCHR_Q > 0:
                s = ps3[:, :, qs:]
                w = SCHR_Q
                y1 = rp.tile([P, 2, w], i16, tag="y1", name="y1")
                y2 = rp.tile([P, 2, w], i16, tag="y2", name="y2")
                nc.vector.tensor_scalar(y1, s, SCHR_K, SCHR_B1, MUL, ADD)
                nc.vector.tensor_scalar(y2, s, SCHR_K, SCHR_B2, MUL, ADD)
                # et = (y2.f16 * W2) + y1.f16
                nc.vector.scalar_tensor_tensor(
                    et3[:, :, qs:], y2.bitcast(f16), SCHR_W2, y1.bitcast(f16),
                    MUL, ADD,
                )

        def emit_combine(prev):
            """Normalize both groups, differential combine, RMSNorm.
            og0/og1 are SBUF [128, 4, 66] f32; outputs on [128, 256] f16."""
            og0l, og1l, _qb, onl = prev
            for qt in range(NQT):
                og0, og1 = og0l[qt], og1l[qt]
                r1 = rp.tile([P, 4], f32, tag="r", name="r1")
                r2 = rp.tile([P, 4], f32, tag="r", name="r2")
                nc.vector.reciprocal_approx_fast(r1, og0[:, :, VD])
                nc.vector.reciprocal_approx_fast(r2, og1[:, :, VD + 1])
                t1 = sqp.tile([P, 4, VD], f32, tag="t1", name="t1", bufs=4)
                o = sqp.tile([P, 4, VD], f32, tag="o", name="o", bufs=4)
                for h in range(4):
                    nc.gpsimd.tensor_scalar_mul(
                        t1[:, h], og0[:, h, 0:VD], r1[:, h:h + 1]
                    )
                    nc.gpsimd.tensor_scalar_mul(
                        o[:, h], og1[:, h, 0:VD], r2[:, h:h + 1]
                    )
                nc.gpsimd.tensor_add(o, o, t1)
                sq = sqp.tile([P, 4, VD], f32, tag="sq", name="sq")
                nc.gpsimd.tensor_mul(sq, o, o)
                ms = rp.tile([P, 4], f32, tag="r", name="ms")
                nc.vector.tensor_reduce(ms, sq, mybir.AxisListType.X, ADD)
                lt = rp.tile([P, 4], f32, tag="r", name="lt")
                nc.scalar.activation(lt, ms, AF.Ln, bias=eps_sb, scale=1.0 / VD)
                rstd = rp.tile([P, 4], f32, tag="r", name="rstd")
                nc.scalar.activation(rstd, lt, AF.Exp, scale=-0.5)
                on = onp.tile([P, 4, VD], f16, tag="on", name="on")
                for h in range(4):
                    nc.gpsimd.tensor_scalar_mul(on[:, h], o[:, h], rstd[:, h:h + 1])
                onl.append(on)

        def emit_transposes(prev):
            onl = prev[3]
            otl = []
            for qt in range(NQT):
                pt = ps_s.tile([P, 2, P], f16, tag="s", name="pt")
                on2 = onl[qt].rearrange("p h v -> p (h v)")
                for c in range(2):
                    nc.tensor.transpose(
                        pt[:, c], on2[:, c * P:(c + 1) * P], ident_sb
                    )
                ot = otp.tile([P, 2, P], f16, tag="ot", name="ot")
                nc.vector.tensor_copy(ot, pt)
                otl.append(ot)
            prev.append(otl)

        def emit_proj(prev, qt):
            _, _, pqb, _, otl = prev
            ot = otl[qt]
            pp = ps_s.tile([P, DIM], f32, tag="s", name="pp")
            for nck in range(2):
                for c in range(2):
                    nc.tensor.matmul(
                        pp[:, nck * QB:(nck + 1) * QB],
                        lhsT=ot[:, c],
                        rhs=wp_sb[:, c, nck * QB:(nck + 1) * QB],
                        start=(c == 0),
                        stop=(c == 1),
                        skip_group_check=True,
                    )
            st = stage.tile([P, DIM], f32, tag="st", name="st")
            nc.vector.tensor_copy(st, pp)
            r0 = pqb * QB + qt * P
            nc.sync.dma_start(out_d[r0:r0 + P, :], st)

        prev = None
        for qb in range(NQB):
            ogs = []
            for g in range(2):
                po = [
                    ps_av.tile([P, QB], f32, tag="av", name=f"po{qt}")
                    for qt in range(NQT)
                ]
                for kt in range(NKT):
                    if g == 0 and prev is not None:
                        if kt == 2:
                            emit_combine(prev)
                        elif kt == 4:
                            emit_transposes(prev)
                        elif kt in (6, 8, 10, 12):
                            emit_proj(prev, (kt - 6) // 2)
                        elif kt == 14:
                            prev = None
                    for h in range(2):
                        ps = ps_s.tile([P, 2 * QB], f32, tag="s", name="ps")
                        for jj in range(2):
                            j = 2 * h + jj
                            nc.tensor.matmul(
                                ps[:, jj * QB:(jj + 1) * QB],
                                lhsT=qk_sb[2 + g][kt // NQB][
                                    HD * j:HD * (j + 1),
                                    (kt % NQB) * P:(kt % NQB + 1) * P,
                                ],
                                rhs=qk_sb[g][qb][HD * j:HD * (j + 1), :],
                                start=True,
                                stop=True,
                                tile_position=(HD * j, 0),
                            )
                        et = etp.tile([P, 2 * QB], f16, tag="e", name="et")
                        emit_exp(et, ps)
                        for jj in range(2):
                            j = 2 * h + jj
                            for qt in range(NQT):
                                nc.tensor.matmul(
                                    po[qt][:, j * VD2:(j + 1) * VD2],
                                    lhsT=et[:, jj * QB + qt * P:jj * QB + (qt + 1) * P],
                                    rhs=vx_sb[kt][:, j, :],
                                    start=(kt == 0 and j == 0),
                                    stop=(kt == NKT - 1 and j == 3),
                                    skip_group_check=True,
                                )
                ogl = []
                for qt in range(NQT):
                    og = ogp.tile([P, 4, VD2], f32, tag="og", name=f"og{g}_{qt}")
                    nc.vector.tensor_copy(
                        og, po[qt][:, :4 * VD2].rearrange("p (j v) -> p j v", j=4)
                    )
                    ogl.append(og)
                ogs.append(ogl)
            prev = [ogs[0], ogs[1], qb, []]

        # tail: last q-block
        emit_combine(prev)
        emit_transposes(prev)
        for qt in range(NQT):
            emit_proj(prev, qt)

    nc.compile()
    return nc


def _get_module():
    if "nc" not in _CACHE:
        _CACHE["nc"] = _build_module()
    return _CACHE["nc"]


def make_in_maps(inputs: dict) -> list:
    x = np.asarray(inputs["x"], np.float32)
    wqkv = np.asarray(inputs["W_qkv"], np.float32)
    wproj = np.asarray(inputs["W_proj"], np.float32)
    lq1 = np.asarray(inputs["lambda_q1"], np.float32)
    lk1 = np.asarray(inputs["lambda_k1"], np.float32)
    lq2 = np.asarray(inputs["lambda_q2"], np.float32)
    lk2 = np.asarray(inputs["lambda_k2"], np.float32)
    subw = np.asarray(inputs["subln_w"], np.float32)

    lam = float(
        np.exp(np.sum(lq1 * lk1)) - np.exp(np.sum(lq2 * lk2)) + LAMBDA_INIT
    )
    vcols = np.empty((P, NKT * 8), np.float16)
    vcols[:, 0::2] = np.float16(1.0)
    vcols[:, 1::2] = np.float16(-1.0 / lam)
    ident = np.eye(P, dtype=np.float16)
    wp_rowscale = (np.tile(subw, 4) * (1.0 - LAMBDA_INIT)).astype(np.float32)

    in_maps = []
    for c in range(NCORES):
        b, g = divmod(c, 4)
        xT = np.ascontiguousarray(x[b].T).astype(np.float16)
        ws = np.ascontiguousarray(
            np.concatenate(
                [
                    wqkv[:, 128 * g:128 * g + 128] * SCALE,
                    wqkv[:, 512 + 128 * g:512 + 128 * g + 128] * SCALE,
                    wqkv[:, 1024 + 128 * g:1024 + 128 * g + 128],
                    wqkv[:, 1536 + 128 * g:1536 + 128 * g + 128],
                    wqkv[:, 2048 + 256 * g:2048 + 256 * g + 256],
                ],
                axis=1,
            )
        ).astype(np.float16)
        wp = (wproj[256 * g:256 * (g + 1), :] * wp_rowscale[:, None]).astype(
            np.float16
        )
        # [256, 1024] -> [128, 2, 1024]: chunk c covers vd rows c*128..c*128+128
        wpd = np.ascontiguousarray(
            wp.reshape(2, P, DIM).transpose(1, 0, 2).reshape(P, 2 * DIM)
        )
        in_maps.append(
            {"xt": xT, "wqkv": ws, "wproj": wpd, "vcols": vcols, "ident": ident}
        )
    return in_maps


def combine_outputs(inputs: dict, parts: list) -> np.ndarray:
    bproj = np.asarray(inputs["b_proj"], np.float32)
    out = np.stack(
        [
            parts[0] + parts[1] + parts[2] + parts[3],
            parts[4] + parts[5] + parts[6] + parts[7],
        ]
    )
    return (out + bproj[None, None, :]).astype(np.float32)


def kernel(**inputs) -> np.ndarray:
    from concourse import bass_utils

    nc = _get_module()
    in_maps = make_in_maps(inputs)
    res = bass_utils.run_bass_kernel_spmd(nc, in_maps, core_ids=list(range(NCORES)))
    parts = [np.asarray(res.results[c]["outp"], np.float32) for c in range(NCORES)]
    return combine_outputs(inputs, parts)


# revision 39
# speedup vs baseline: 1.1312x; 1.1184x over previous
"""DiffAttention Trainium2 kernel (8-core SPMD, full-I/O contract), v2.

Sharding: core c = (batch b = c//4) x (head-group g = c%4, 4 of 16 v-heads).

Key design points (cost-model-driven rewrite of the v1 baseline):
  - All PE inputs are fp16 (1 cycle/row at ANY moving size, vs f32r which
    needs >=256).  End-to-end rel err ~6e-4 (measured in numpy), budget 2e-2.
  - Scores are computed transposed (S^T [ktok 128, qtok 512]) as before, but
    the AV matmul is flipped to out[q, vd]: lhsT = expS^T chunk [128k, 128q],
    rhs = v-head [128k, 66] -> ap=66 per matmul instead of 512.  This cuts
    the AV stream time ~7.8x (cost = moving dim only).
  - v tiles carry TWO extra columns: ones (group-1 softmax denominator) and
    -1/lambda (group-2), so both AV accumulators come out of PSUM with their
    reciprocal-ready denominators at cols 64/65 of each 66-block.
  - Softmax normalization, differential combine and RMSNorm all happen in
    the natural [q, vd] orientation: per-partition-scalar ops on Pool/DVE,
    no partition broadcasts, no GPSIMD reductions.
  - o_n is transposed back with two PE transpose ops per q-tile (ap=128) and
    projected with contract=128 (2 passes instead of 4): proj drops 2x.
  - The exp softmax (the ACT bottleneck, 256 x [128,1024] tiles) can be
    split by q-columns between ACT (table exp) and DVE (dual-Schraudolph
    bit-trick exp, 3 ops/tile-slice, rel rms ~0.5%).  Rows stay pure so the
    systematic part cancels in the softmax normalization.  SCHR_Q controls
    the number of q-columns (per 512-block) done on DVE.
  - combine/proj of q-block N are software-pipelined into q-block N+1's
    group-0 attention stream (emission order = engine order).
"""

import numpy as np

P = 128
N_TOK = 2048
DIM = 1024
NCORES = 8
HD = 32
VD = 64            # v-head dim
VD2 = VD + 2       # + ones column (g0 denom) + (-1/lam) column (g1 denom)
NKD = DIM // P     # 8 k-chunks over the model dim
NKT = N_TOK // P   # 16 token tiles
QB = 512           # query block
NQB = N_TOK // QB  # 4
NQT = QB // P      # 4 q-tiles per block
LAMBDA_INIT = 0.8 - 0.6 * float(np.exp(-0.3 * 12))
EPS = 1e-5
SCALE = HD ** -0.5

# exp split: number of q-columns (of each 512-q block) computed on DVE via
# dual-Schraudolph.  0 = everything on ACT.
SCHR_Q = 0
# dual-Schraudolph constants (fp16 bit trick, see docstring)
SCHR_K = float(1.4426950408889634 * 1024.0)
SCHR_C = 0.03
SCHR_B1 = float(15360.0 + 0.5 - SCHR_C * 1024.0 - 1024.0)
SCHR_B2 = float(15360.0 + 0.5 - SCHR_C * 1024.0 - 512.0)
SCHR_W2 = float(np.sqrt(2.0) / 2.0)

_CACHE: dict = {}


def _build_module():
    from contextlib import ExitStack

    import concourse.bass as bass  # noqa: F401
    import concourse.mybir as mybir
    import concourse.tile as tile
    from concourse import bacc

    f32 = mybir.dt.float32
    f16 = mybir.dt.float16
    i16 = mybir.dt.int16
    AF = mybir.ActivationFunctionType
    MUL = mybir.AluOpType.mult
    ADD = mybir.AluOpType.add

    nc = bacc.Bacc(
        "TRN2", target_bir_lowering=False, debug=False, num_devices=NCORES
    )

    # Pin the one ACT table set that contains every function we use
    # (exp, ln, copy, square).  Without this the table-load pass ping-pongs
    # between "exp_and_others" and "natural_log" on every RMSNorm (1.3us +
    # pipeline stall per swap).
    from concourse.hw_specs import get_activation_tables

    _tables = get_activation_tables(nc.m.arch)
    _need = {AF.Exp, AF.Ln, AF.Copy, AF.Square, AF.Identity}
    _set_id = next(
        i for i, (_n, funcs) in enumerate(_tables.items()) if _need <= funcs
    )

    xT_d = nc.dram_tensor("xt", [DIM, N_TOK], f16, kind="ExternalInput").ap()
    wqkv_d = nc.dram_tensor("wqkv", [DIM, 768], f16, kind="ExternalInput").ap()
    wproj_d = nc.dram_tensor("wproj", [P, 2 * DIM], f16, kind="ExternalInput").ap()
    vcols_d = nc.dram_tensor("vcols", [P, NKT * 8], f16, kind="ExternalInput").ap()
    ident_d = nc.dram_tensor("ident", [P, P], f16, kind="ExternalInput").ap()
    out_d = nc.dram_tensor("outp", [N_TOK, DIM], f32, kind="ExternalOutput").ap()

    with ExitStack() as ctx:
        tc = ctx.enter_context(tile.TileContext(nc))

        singles = ctx.enter_context(tc.tile_pool(name="singles", bufs=1))
        ps_s = ctx.enter_context(tc.tile_pool(name="ps_s", bufs=2, space="PSUM"))
        ps_av = ctx.enter_context(tc.tile_pool(name="ps_av", bufs=4, space="PSUM"))
        etp = ctx.enter_context(tc.tile_pool(name="etp", bufs=8))
        ogp = ctx.enter_context(tc.tile_pool(name="ogp", bufs=8))
        onp = ctx.enter_context(tc.tile_pool(name="onp", bufs=4))
        rp = ctx.enter_context(tc.tile_pool(name="rp", bufs=24))
        sqp = ctx.enter_context(tc.tile_pool(name="sqp", bufs=2))
        otp = ctx.enter_context(tc.tile_pool(name="otp", bufs=4))
        stage = ctx.enter_context(tc.tile_pool(name="stage", bufs=3))

        # qk tiles: [m][n] -> (x @ Wm)^T chunk, m in (q1, q2, k1, k2), n = tok/512
        qk_sb = [
            [singles.tile([P, QB], f16, tag=f"qk{m}_{n}", name=f"qk{m}_{n}") for n in range(NQB)]
            for m in range(4)
        ]
        vx_sb = [singles.tile([P, 4, VD2], f16, tag=f"vx{t}", name=f"vx{t}") for t in range(NKT)]
        wp_sb = singles.tile([P, 2, DIM], f16, tag="wp")
        ident_sb = singles.tile([P, P], f16, tag="id")
        eps_sb = singles.tile([P, 1], f32, tag="eps")

        nc.vector.memset(eps_sb, EPS)
        nc.sync.dma_start(ident_sb, ident_d)
        nc.sync.dma_start(wp_sb, wproj_d.rearrange("p (c n) -> p c n", c=2))

        # ---- stage 1: qkv projections ----
        # Only the prefix needed by (qb0, g0) runs before attention starts:
        # q1[n0], all k1, all v.  The remaining 11 qk groups are injected
        # into qb0's attention stream (one per odd kt) where the PE has
        # slack under the ACT-bound exp cadence.
        ph1 = ctx.enter_context(tc.tile_pool(name="ph1", bufs=1))
        xT_t = xT_d.rearrange("(ko p) t -> ko p t", p=P)
        wq_t = wqkv_d.rearrange("(ko p) c -> ko p c", p=P)
        x_sb = []
        w_sb = []
        for k in range(NKD):
            wt = ph1.tile([P, 768], f16, tag=f"w{k}", name=f"w{k}")
            nc.sync.dma_start(wt, wq_t[k])
            w_sb.append(wt)
            xt = ph1.tile([P, N_TOK], f16, tag=f"x{k}", name=f"x{k}")
            # token-block n0 lands with the weights so the first qk
            # groups start ~8us earlier; later blocks chase.
            nc.sync.dma_start(xt[:, :QB], xT_t[k][:, :QB])
            x_sb.append(xt)
        for n in range(1, NQB):
            for k in range(NKD):
                nc.sync.dma_start(
                    x_sb[k][:, n * QB:(n + 1) * QB],
                    xT_t[k][:, n * QB:(n + 1) * QB],
                )
        for t in range(NKT):
            nc.sync.dma_start(
                vx_sb[t][:, :, VD:VD2],
                vcols_d.rearrange("p (t j c) -> p t j c", j=4, c=2)[:, t],
            )

        def qk_group(m, n, eng, pool, tag):
            pqk = pool.tile([P, QB], f32, tag=tag, name="s1qk")
            for k in range(NKD):
                nc.tensor.matmul(
                    pqk,
                    lhsT=w_sb[k][:, m * P:(m + 1) * P],
                    rhs=x_sb[k][:, n * QB:(n + 1) * QB],
                    start=(k == 0),
                    stop=(k == NKD - 1),
                )
            if eng == 0:
                nc.vector.tensor_copy(qk_sb[m][n], pqk)
            else:
                nc.scalar.copy(qk_sb[m][n], pqk)

        def qk_chunk(m, n, half):
            """Half of a qk group (4 of 8 contraction chunks) as a
            self-contained transient: fits the per-kt PE slack under the
            exp cadence.  half 0 copies, half 1 accumulates via DVE add."""
            pqk = ps_s.tile([P, QB], f32, tag="s", name="s1qkc")
            for kk in range(4):
                k = 4 * half + kk
                nc.tensor.matmul(
                    pqk,
                    lhsT=w_sb[k][:, m * P:(m + 1) * P],
                    rhs=x_sb[k][:, n * QB:(n + 1) * QB],
                    start=(kk == 0),
                    stop=(kk == 3),
                )
            if half == 0:
                nc.vector.tensor_copy(qk_sb[m][n], pqk)
            else:
                nc.vector.tensor_add(qk_sb[m][n], pqk, qk_sb[m][n])

        def v_chunk(i):
            v_group(i, 0, pool=ps_s, tag="s")

        def v_group(i, eng, pool=None, tag="av"):
            pool = pool or ps_av
            pv = pool.tile([P, QB], f32, tag=tag, name="s1v")[:, :4 * VD]
            for k in range(NKD):
                nc.tensor.matmul(
                    pv,
                    lhsT=x_sb[k][:, i * P:(i + 1) * P],
                    rhs=w_sb[k][:, 512:768],
                    start=(k == 0),
                    stop=(k == NKD - 1),
                )
            dst = vx_sb[i][:, :, 0:VD]
            src = pv.rearrange("p (j v) -> p j v", j=4)
            if eng == 0:
                nc.vector.tensor_copy(dst, src)
            else:
                nc.scalar.copy(dst, src)

        INJECT = True
        # minimal prefix for (qb0, g0, kt0..7): q1[n0], k1[n0], k1[n1], v0-7
        qk_group(0, 0, 0, ps_av, "av")
        qk_group(2, 0, 1, ps_av, "av")
        for i in range(4):
            v_group(i, i % 2)
        qk_group(2, 1, 1, ps_av, "av")
        for i in range(4, 8):
            v_group(i, i % 2)
        if not INJECT:
            for i in range(8, 16):
                v_group(i, i % 2)
            qk_group(2, 2, 0, ps_av, "av")
            qk_group(2, 3, 1, ps_av, "av")
            qk_group(1, 0, 0, ps_av, "av")
            for n in range(4):
                qk_group(3, n, n % 2, ps_av, "av")
            for n in range(1, NQB):
                qk_group(0, n, n % 2, ps_av, "av")
                qk_group(1, n, (n + 1) % 2, ps_av, "av")
        # everything else is injected one chunk per kt, deadline-ordered.
        # key: (qb, g) -> list of thunks popped one per kt
        inj = {} if not INJECT else {
            (0, 0): [
                lambda: qk_chunk(2, 2, 0), lambda: qk_chunk(2, 2, 1),
                lambda: v_chunk(8), lambda: v_chunk(9), lambda: v_chunk(10),
                lambda: qk_chunk(2, 3, 0), lambda: qk_chunk(2, 3, 1),
                lambda: v_chunk(11), lambda: v_chunk(12), lambda: v_chunk(13),
                lambda: v_chunk(14), lambda: v_chunk(15),
                lambda: qk_chunk(1, 0, 0), lambda: qk_chunk(1, 0, 1),
                lambda: qk_chunk(3, 0, 0), lambda: qk_chunk(3, 0, 1),
            ],
            (0, 1): [
                lambda: qk_chunk(3, 1, 0), lambda: qk_chunk(3, 1, 1),
                lambda: qk_chunk(3, 2, 0), lambda: qk_chunk(3, 2, 1),
                lambda: qk_chunk(3, 3, 0), lambda: qk_chunk(3, 3, 1),
                lambda: qk_chunk(0, 1, 0), lambda: qk_chunk(0, 1, 1),
                lambda: qk_chunk(1, 1, 0), lambda: qk_chunk(1, 1, 1),
                lambda: qk_chunk(0, 2, 0), lambda: qk_chunk(0, 2, 1),
            ],
            (1, 0): [
                lambda: qk_chunk(1, 2, 0), lambda: qk_chunk(1, 2, 1),
                lambda: qk_chunk(0, 3, 0), lambda: qk_chunk(0, 3, 1),
            ],
            (1, 1): [
                lambda: qk_chunk(1, 3, 0), lambda: qk_chunk(1, 3, 1),
            ],
        }

        # ---- attention ----

        def emit_exp(et, ps):
            """expS^T for one head-pair tile: ACT exp on the first
            QB-SCHR_Q q-columns of each 512 block, DVE dual-Schraudolph on
            the rest (rows stay implementation-pure across kt)."""
            qs = QB - SCHR_Q
            et3 = et.rearrange("p (h q) -> p h q", h=2)
            ps3 = ps.rearrange("p (h q) -> p h q", h=2)
            if qs > 0:
                nc.scalar.activation(et3[:, :, :qs], ps3[:, :, :qs], AF.Exp)
            if SCHR_Q > 0:
                s = ps3[:, :, qs:]
                w = SCHR_Q
                y1 = rp.tile([P, 2, w], i16, tag="y1", name="y1")
                y2 = rp.tile([P, 2, w], i16, tag="y2", name="y2")
                nc.vector.tensor_scalar(y1, s, SCHR_K, SCHR_B1, MUL, ADD)
                nc.vector.tensor_scalar(y2, s, SCHR_K, SCHR_B2, MUL, ADD)
                # et = (y2.f16 * W2) + y1.f16
                nc.vector.scalar_tensor_tensor(
                    et3[:, :, qs:], y2.bitcast(f16), SCHR_W2, y1.bitcast(f16),
                    MUL, ADD,
                )

        def emit_a1(og0, eng=None):
            """Group-1 softmax normalization half of the combine (only needs
            og0, so it can run during the last q-block's g1 attention)."""
            eng = eng or nc.gpsimd
            r1 = rp.tile([P, 4], f32, tag="r", name="r1")
            nc.vector.reciprocal_approx_fast(r1, og0[:, :, VD])
            t1 = sqp.tile([P, 4, VD], f32, tag="t1", name="t1", bufs=4)
            for h in range(4):
                eng.tensor_scalar_mul(t1[:, h], og0[:, h, 0:VD], r1[:, h:h + 1])
            return t1

        def emit_combine_a(prev, qt, eng=None, t1=None):
            """Softmax-normalize both groups, differential combine, and the
            per-head sum-of-squares (fused via scalar_tensor_tensor accum).
            Pool-only math after two DVE reciprocals, so the DVE stream
            never waits on Pool."""
            og0l, og1l, _qb, oml, onl = prev[:5]
            eng = eng or nc.gpsimd
            if t1 is None:
                t1 = emit_a1(og0l[qt], eng)
            og1 = og1l[qt]
            r2 = rp.tile([P, 4], f32, tag="r", name="r2")
            nc.vector.reciprocal_approx_fast(r2, og1[:, :, VD + 1])
            o = sqp.tile([P, 4, VD], f32, tag="o", name="o", bufs=6)
            for h in range(4):
                eng.tensor_scalar_mul(o[:, h], og1[:, h, 0:VD], r2[:, h:h + 1])
            eng.tensor_add(o, o, t1)
            sq = sqp.tile([P, 4, VD], f32, tag="sq", name="sq")
            eng.tensor_mul(sq, o, o)
            ms = rp.tile([P, 4], f32, tag="r", name="ms")
            nc.vector.tensor_reduce(ms, sq, mybir.AxisListType.X, ADD)
            oml.append((o, ms))

        def emit_combine_b(prev, qt, eng=None):
            """rstd = exp(-0.5*ln(ms/64+eps)) on ACT (table already hot),
            apply on Pool."""
            eng = eng or nc.gpsimd
            oml, onl = prev[3], prev[4]
            o, ms = oml[qt]
            lt = rp.tile([P, 4], f32, tag="r", name="lt")
            nc.scalar.activation(lt, ms, AF.Ln, bias=eps_sb, scale=1.0 / VD)
            rstd = rp.tile([P, 4], f32, tag="r", name="rstd")
            nc.scalar.activation(rstd, lt, AF.Exp, scale=-0.5)
            on = onp.tile([P, 4, VD], f16, tag="on", name="on")
            for h in range(4):
                eng.tensor_scalar_mul(on[:, h], o[:, h], rstd[:, h:h + 1])
            onl.append(on)

        def emit_transpose(prev, qt):
            onl, otl = prev[4], prev[5]
            pt = ps_s.tile([P, 2, P], f16, tag="s", name="pt")
            on2 = onl[qt].rearrange("p h v -> p (h v)")
            for c in range(2):
                nc.tensor.transpose(pt[:, c], on2[:, c * P:(c + 1) * P], ident_sb)
            ot = otp.tile([P, 2, P], f16, tag="ot", name="ot")
            nc.vector.tensor_copy(ot, pt)
            otl.append(ot)

        def emit_proj(prev, qt):
            pqb, otl = prev[2], prev[5]
            ot = otl[qt]
            pp = ps_s.tile([P, DIM], f32, tag="s", name="pp")
            for nck in range(2):
                for c in range(2):
                    nc.tensor.matmul(
                        pp[:, nck * QB:(nck + 1) * QB],
                        lhsT=ot[:, c],
                        rhs=wp_sb[:, c, nck * QB:(nck + 1) * QB],
                        start=(c == 0),
                        stop=(c == 1),
                        skip_group_check=True,
                    )
            st = stage.tile([P, DIM], f32, tag="st", name="st")
            nc.vector.tensor_copy(st, pp)
            r0 = pqb * QB + qt * P
            nc.sync.dma_start(out_d[r0:r0 + P, :], st)

        prev = None
        pre_t1 = []
        for qb in range(NQB):
            ogs = []
            for g in range(2):
                po = [
                    ps_av.tile([P, QB], f32, tag="av", name=f"po{qt}")
                    for qt in range(NQT)
                ]

                def emit_av(et, kt, h):
                    for jj in range(2):
                        j = 2 * h + jj
                        for qt in range(NQT):
                            nc.tensor.matmul(
                                po[qt][:, j * VD2:(j + 1) * VD2],
                                lhsT=et[:, jj * QB + qt * P:jj * QB + (qt + 1) * P],
                                rhs=vx_sb[kt][:, j, :],
                                start=(kt == 0 and j == 0),
                                stop=(kt == NKT - 1 and j == 3),
                                skip_group_check=True,
                            )

                pend = []
                thunks = inj.get((qb, g), [])
                for kt in range(NKT):
                    if thunks:
                        thunks.pop(0)()
                    if qb == NQB - 1 and g == 1 and kt in (2, 3, 4, 5):
                        pre_t1.append(emit_a1(ogs[0][kt - 2]))
                    if g == 0 and prev is not None:
                        if kt in (1, 2, 3, 4):
                            emit_combine_a(prev, kt - 1)
                        elif kt in (6, 7):
                            emit_combine_b(prev, 2 * (kt - 6))
                            emit_combine_b(prev, 2 * (kt - 6) + 1)
                        elif kt == 8:
                            for qt in range(NQT):
                                emit_transpose(prev, qt)
                        elif kt in (9, 10, 11, 12):
                            emit_proj(prev, kt - 9)
                        elif kt == 13:
                            prev = None
                    for h in range(2):
                        ps = ps_s.tile([P, 2 * QB], f32, tag="s", name="ps")
                        for jj in range(2):
                            j = 2 * h + jj
                            nc.tensor.matmul(
                                ps[:, jj * QB:(jj + 1) * QB],
                                lhsT=qk_sb[2 + g][kt // NQB][
                                    HD * j:HD * (j + 1),
                                    (kt % NQB) * P:(kt % NQB + 1) * P,
                                ],
                                rhs=qk_sb[g][qb][HD * j:HD * (j + 1), :],
                                start=True,
                                stop=True,
                                tile_position=(HD * j, 0),
                            )
                        et = etp.tile([P, 2 * QB], f16, tag="e", name="et")
                        emit_exp(et, ps)
                        pend.append((et, kt, h))
                    # software pipeline: AV for tile kt runs after the
                    # scores for kt+1 are on the PE queue, so ACT always has
                    # its next input ready and the PE never gates it.
                    while len(pend) > 2:
                        emit_av(*pend.pop(0))
                for item in pend:
                    emit_av(*item)
                ogl = []
                for qt in range(NQT):
                    og = ogp.tile([P, 4, VD2], f32, tag="og", name=f"og{g}_{qt}")
                    nc.vector.tensor_copy(
                        og, po[qt][:, :4 * VD2].rearrange("p (j v) -> p j v", j=4)
                    )
                    ogl.append(og)
                ogs.append(ogl)
            prev = [ogs[0], ogs[1], qb, [], [], []]

        # tail: last q-block, per-qt pipelined across DVE/Pool/ACT/PE
        engs = [nc.gpsimd, nc.vector, nc.gpsimd, nc.vector]
        for qt in range(NQT):
            emit_combine_a(prev, qt, eng=engs[qt], t1=pre_t1[qt])
        for qt in range(NQT):
            emit_combine_b(prev, qt, eng=engs[qt])
            emit_transpose(prev, qt)
            emit_proj(prev, qt)

    nc.compile()
    return nc


def _get_module():
    if "nc" not in _CACHE:
        _CACHE["nc"] = _build_module()
    return _CACHE["nc"]


def make_in_maps(inputs: dict) -> list:
    x = np.asarray(inputs["x"], np.float32)
    wqkv = np.asarray(inputs["W_qkv"], np.float32)
    wproj = np.asarray(inputs["W_proj"], np.float32)
    lq1 = np.asarray(inputs["lambda_q1"], np.float32)
    lk1 = np.asarray(inputs["lambda_k1"], np.float32)
    lq2 = np.asarray(inputs["lambda_q2"], np.float32)
    lk2 = np.asarray(inputs["lambda_k2"], np.float32)
    subw = np.asarray(inputs["subln_w"], np.float32)

    lam = float(
        np.exp(np.sum(lq1 * lk1)) - np.exp(np.sum(lq2 * lk2)) + LAMBDA_INIT
    )
    vcols = np.empty((P, NKT * 8), np.float16)
    vcols[:, 0::2] = np.float16(1.0)
    vcols[:, 1::2] = np.float16(-1.0 / lam)
    ident = np.eye(P, dtype=np.float16)
    wp_rowscale = (np.tile(subw, 4) * (1.0 - LAMBDA_INIT)).astype(np.float32)

    in_maps = []
    for c in range(NCORES):
        b, g = divmod(c, 4)
        xT = np.ascontiguousarray(x[b].T).astype(np.float16)
        ws = np.ascontiguousarray(
            np.concatenate(
                [
                    wqkv[:, 128 * g:128 * g + 128] * SCALE,
                    wqkv[:, 512 + 128 * g:512 + 128 * g + 128] * SCALE,
                    wqkv[:, 1024 + 128 * g:1024 + 128 * g + 128],
                    wqkv[:, 1536 + 128 * g:1536 + 128 * g + 128],
                    wqkv[:, 2048 + 256 * g:2048 + 256 * g + 256],
                ],
                axis=1,
            )
        ).astype(np.float16)
        wp = (wproj[256 * g:256 * (g + 1), :] * wp_rowscale[:, None]).astype(
            np.float16
        )
        # [256, 1024] -> [128, 2, 1024]: chunk c covers vd rows c*128..c*128+128
        wpd = np.ascontiguousarray(
            wp.reshape(2, P, DIM).transpose(1, 0, 2).reshape(P, 2 * DIM)
        )
        in_maps.append(
            {"xt": xT, "wqkv": ws, "wproj": wpd, "vcols": vcols, "ident": ident}
        )
    return in_maps


def combine_outputs(inputs: dict, parts: list) -> np.ndarray:
    bproj = np.asarray(inputs["b_proj"], np.float32)
    out = np.stack(
        [
            parts[0] + parts[1] + parts[2] + parts[3],
            parts[4] + parts[5] + parts[6] + parts[7],
        ]
    )
    return (out + bproj[None, None, :]).astype(np.float32)


def kernel(**inputs) -> np.ndarray:
    from concourse import bass_utils

    nc = _get_module()
    in_maps = make_in_maps(inputs)
    res = bass_utils.run_bass_kernel_spmd(nc, in_maps, core_ids=list(range(NCORES)))
    parts = [np.asarray(res.results[c]["outp"], np.float32) for c in range(NCORES)]
    return combine_outputs(inputs, parts)


# revision 40
# speedup vs baseline: 1.1959x; 1.0572x over previous
"""DiffAttention Trainium2 kernel (8-core SPMD, full-I/O contract), v2.

Sharding: core c = (batch b = c//4) x (head-group g = c%4, 4 of 16 v-heads).

Key design points (cost-model-driven rewrite of the v1 baseline):
  - All PE inputs are fp16 (1 cycle/row at ANY moving size, vs f32r which
    needs >=256).  End-to-end rel err ~6e-4 (measured in numpy), budget 2e-2.
  - Scores are computed transposed (S^T [ktok 128, qtok 512]) as before, but
    the AV matmul is flipped to out[q, vd]: lhsT = expS^T chunk [128k, 128q],
    rhs = v-head [128k, 66] -> ap=66 per matmul instead of 512.  This cuts
    the AV stream time ~7.8x (cost = moving dim only).
  - v tiles carry TWO extra columns: ones (group-1 softmax denominator) and
    -1/lambda (group-2), so both AV accumulators come out of PSUM with their
    reciprocal-ready denominators at cols 64/65 of each 66-block.
  - Softmax normalization, differential combine and RMSNorm all happen in
    the natural [q, vd] orientation: per-partition-scalar ops on Pool/DVE,
    no partition broadcasts, no GPSIMD reductions.
  - o_n is transposed back with two PE transpose ops per q-tile (ap=128) and
    projected with contract=128 (2 passes instead of 4): proj drops 2x.
  - The exp softmax (the ACT bottleneck, 256 x [128,1024] tiles) can be
    split by q-columns between ACT (table exp) and DVE (dual-Schraudolph
    bit-trick exp, 3 ops/tile-slice, rel rms ~0.5%).  Rows stay pure so the
    systematic part cancels in the softmax normalization.  SCHR_Q controls
    the number of q-columns (per 512-block) done on DVE.
  - combine/proj of q-block N are software-pipelined into q-block N+1's
    group-0 attention stream (emission order = engine order).
"""

import numpy as np

P = 128
N_TOK = 2048
DIM = 1024
NCORES = 8
HD = 32
VD = 64            # v-head dim
VD2 = VD + 2       # + ones column (g0 denom) + (-1/lam) column (g1 denom)
NKD = DIM // P     # 8 k-chunks over the model dim
NKT = N_TOK // P   # 16 token tiles
QB = 512           # query block
NQB = N_TOK // QB  # 4
NQT = QB // P      # 4 q-tiles per block
LAMBDA_INIT = 0.8 - 0.6 * float(np.exp(-0.3 * 12))
EPS = 1e-5
SCALE = HD ** -0.5

# exp split: number of q-columns (of each 512-q block) computed on DVE via
# dual-Schraudolph.  0 = everything on ACT.
SCHR_Q = 0
# dual-Schraudolph constants (fp16 bit trick, see docstring)
SCHR_K = float(1.4426950408889634 * 1024.0)
SCHR_C = 0.03
SCHR_B1 = float(15360.0 + 0.5 - SCHR_C * 1024.0 - 1024.0)
SCHR_B2 = float(15360.0 + 0.5 - SCHR_C * 1024.0 - 512.0)
SCHR_W2 = float(np.sqrt(2.0) / 2.0)

_CACHE: dict = {}


def _build_module():
    from contextlib import ExitStack

    import concourse.bass as bass  # noqa: F401
    import concourse.mybir as mybir
    import concourse.tile as tile
    from concourse import bacc

    f32 = mybir.dt.float32
    f16 = mybir.dt.float16
    i16 = mybir.dt.int16
    AF = mybir.ActivationFunctionType
    MUL = mybir.AluOpType.mult
    ADD = mybir.AluOpType.add

    nc = bacc.Bacc(
        "TRN2", target_bir_lowering=False, debug=False, num_devices=NCORES
    )

    # Pin the one ACT table set that contains every function we use
    # (exp, ln, copy, square).  Without this the table-load pass ping-pongs
    # between "exp_and_others" and "natural_log" on every RMSNorm (1.3us +
    # pipeline stall per swap).
    from concourse.hw_specs import get_activation_tables

    _tables = get_activation_tables(nc.m.arch)
    _need = {AF.Exp, AF.Ln, AF.Copy, AF.Square, AF.Identity}
    _set_id = next(
        i for i, (_n, funcs) in enumerate(_tables.items()) if _need <= funcs
    )

    xT_d = nc.dram_tensor("xt", [DIM, N_TOK], f16, kind="ExternalInput").ap()
    wqkv_d = nc.dram_tensor("wqkv", [DIM, 768], f16, kind="ExternalInput").ap()
    wproj_d = nc.dram_tensor("wproj", [P, 2 * DIM], f16, kind="ExternalInput").ap()
    vcols_d = nc.dram_tensor("vcols", [P, NKT * 8], f16, kind="ExternalInput").ap()
    ident_d = nc.dram_tensor("ident", [P, P], f16, kind="ExternalInput").ap()
    out_d = nc.dram_tensor("outp", [N_TOK, DIM], f32, kind="ExternalOutput").ap()

    with ExitStack() as ctx:
        tc = ctx.enter_context(tile.TileContext(nc))

        singles = ctx.enter_context(tc.tile_pool(name="singles", bufs=1))
        ps_s = ctx.enter_context(tc.tile_pool(name="ps_s", bufs=2, space="PSUM"))
        ps_av = ctx.enter_context(tc.tile_pool(name="ps_av", bufs=4, space="PSUM"))
        etp = ctx.enter_context(tc.tile_pool(name="etp", bufs=8))
        ogp = ctx.enter_context(tc.tile_pool(name="ogp", bufs=8))
        onp = ctx.enter_context(tc.tile_pool(name="onp", bufs=4))
        rp = ctx.enter_context(tc.tile_pool(name="rp", bufs=24))
        sqp = ctx.enter_context(tc.tile_pool(name="sqp", bufs=2))
        otp = ctx.enter_context(tc.tile_pool(name="otp", bufs=4))
        stage = ctx.enter_context(tc.tile_pool(name="stage", bufs=3))

        # qk tiles: [m][n] -> (x @ Wm)^T chunk, m in (q1, q2, k1, k2), n = tok/512
        qk_sb = [
            [singles.tile([P, QB], f16, tag=f"qk{m}_{n}", name=f"qk{m}_{n}") for n in range(NQB)]
            for m in range(4)
        ]
        vx_sb = [singles.tile([P, 4, VD2], f16, tag=f"vx{t}", name=f"vx{t}") for t in range(NKT)]
        wp_sb = singles.tile([P, 2, DIM], f16, tag="wp")
        ident_sb = singles.tile([P, P], f16, tag="id")
        eps_sb = singles.tile([P, 1], f32, tag="eps")

        nc.vector.memset(eps_sb, EPS)
        nc.sync.dma_start(ident_sb, ident_d)
        nc.sync.dma_start(wp_sb, wproj_d.rearrange("p (c n) -> p c n", c=2))

        # ---- stage 1: qkv projections ----
        # Only the prefix needed by (qb0, g0) runs before attention starts:
        # q1[n0], all k1, all v.  The remaining 11 qk groups are injected
        # into qb0's attention stream (one per odd kt) where the PE has
        # slack under the ACT-bound exp cadence.
        ph1 = ctx.enter_context(tc.tile_pool(name="ph1", bufs=1))
        xT_t = xT_d.rearrange("(ko p) t -> ko p t", p=P)
        wq_t = wqkv_d.rearrange("(ko p) c -> ko p c", p=P)
        x_sb = []
        w_sb = []
        for k in range(NKD):
            wt = ph1.tile([P, 768], f16, tag=f"w{k}", name=f"w{k}")
            nc.sync.dma_start(wt, wq_t[k])
            w_sb.append(wt)
            xt = ph1.tile([P, N_TOK], f16, tag=f"x{k}", name=f"x{k}")
            # token-block n0 lands with the weights so the first qk
            # groups start ~8us earlier; later blocks chase.
            nc.sync.dma_start(xt[:, :QB], xT_t[k][:, :QB])
            x_sb.append(xt)
        for n in range(1, NQB):
            for k in range(NKD):
                nc.sync.dma_start(
                    x_sb[k][:, n * QB:(n + 1) * QB],
                    xT_t[k][:, n * QB:(n + 1) * QB],
                )
        for t in range(NKT):
            nc.sync.dma_start(
                vx_sb[t][:, :, VD:VD2],
                vcols_d.rearrange("p (t j c) -> p t j c", j=4, c=2)[:, t],
            )

        def qk_group(m, n, eng, pool, tag):
            pqk = pool.tile([P, QB], f32, tag=tag, name="s1qk")
            for k in range(NKD):
                nc.tensor.matmul(
                    pqk,
                    lhsT=w_sb[k][:, m * P:(m + 1) * P],
                    rhs=x_sb[k][:, n * QB:(n + 1) * QB],
                    start=(k == 0),
                    stop=(k == NKD - 1),
                )
            if eng == 0:
                nc.vector.tensor_copy(qk_sb[m][n], pqk)
            else:
                nc.scalar.copy(qk_sb[m][n], pqk)

        def qk_chunk(m, n, half):
            """Half of a qk group (4 of 8 contraction chunks) as a
            self-contained transient: fits the per-kt PE slack under the
            exp cadence.  half 0 copies, half 1 accumulates via DVE add."""
            pqk = ps_s.tile([P, QB], f32, tag="s", name="s1qkc")
            for kk in range(4):
                k = 4 * half + kk
                nc.tensor.matmul(
                    pqk,
                    lhsT=w_sb[k][:, m * P:(m + 1) * P],
                    rhs=x_sb[k][:, n * QB:(n + 1) * QB],
                    start=(kk == 0),
                    stop=(kk == 3),
                )
            if half == 0:
                nc.vector.tensor_copy(qk_sb[m][n], pqk)
            else:
                nc.vector.tensor_add(qk_sb[m][n], pqk, qk_sb[m][n])

        def v_chunk(i):
            v_group(i, 0, pool=ps_s, tag="s")

        def v_group(i, eng, pool=None, tag="av"):
            pool = pool or ps_av
            pv = pool.tile([P, QB], f32, tag=tag, name="s1v")[:, :4 * VD]
            for k in range(NKD):
                nc.tensor.matmul(
                    pv,
                    lhsT=x_sb[k][:, i * P:(i + 1) * P],
                    rhs=w_sb[k][:, 512:768],
                    start=(k == 0),
                    stop=(k == NKD - 1),
                )
            dst = vx_sb[i][:, :, 0:VD]
            src = pv.rearrange("p (j v) -> p j v", j=4)
            if eng == 0:
                nc.vector.tensor_copy(dst, src)
            else:
                nc.scalar.copy(dst, src)

        INJECT = True
        # minimal prefix for (qb0, g0, kt0..7): q1[n0], k1[n0], k1[n1], v0-7
        qk_group(0, 0, 0, ps_av, "av")
        qk_group(2, 0, 1, ps_av, "av")
        for i in range(4):
            v_group(i, i % 2)
        qk_group(2, 1, 1, ps_av, "av")
        for i in range(4, 8):
            v_group(i, i % 2)
        if not INJECT:
            for i in range(8, 16):
                v_group(i, i % 2)
            qk_group(2, 2, 0, ps_av, "av")
            qk_group(2, 3, 1, ps_av, "av")
            qk_group(1, 0, 0, ps_av, "av")
            for n in range(4):
                qk_group(3, n, n % 2, ps_av, "av")
            for n in range(1, NQB):
                qk_group(0, n, n % 2, ps_av, "av")
                qk_group(1, n, (n + 1) % 2, ps_av, "av")
        # everything else is injected one chunk per kt, deadline-ordered.
        # key: (qb, g) -> list of thunks popped one per kt
        inj = {} if not INJECT else {
            (0, 0): [
                lambda: qk_chunk(2, 2, 0), lambda: qk_chunk(2, 2, 1),
                lambda: v_chunk(8), lambda: v_chunk(9), lambda: v_chunk(10),
                lambda: qk_chunk(2, 3, 0), lambda: qk_chunk(2, 3, 1),
                lambda: v_chunk(11), lambda: v_chunk(12), lambda: v_chunk(13),
                lambda: v_chunk(14), lambda: v_chunk(15),
                lambda: qk_chunk(1, 0, 0), lambda: qk_chunk(1, 0, 1),
                lambda: qk_chunk(3, 0, 0), lambda: qk_chunk(3, 0, 1),
            ],
            (0, 1): [
                lambda: qk_chunk(3, 1, 0), lambda: qk_chunk(3, 1, 1),
                lambda: qk_chunk(3, 2, 0), lambda: qk_chunk(3, 2, 1),
                lambda: qk_chunk(3, 3, 0), lambda: qk_chunk(3, 3, 1),
                lambda: qk_chunk(0, 1, 0), lambda: qk_chunk(0, 1, 1),
                lambda: qk_chunk(1, 1, 0), lambda: qk_chunk(1, 1, 1),
                lambda: qk_chunk(0, 2, 0), lambda: qk_chunk(0, 2, 1),
            ],
            (1, 0): [
                lambda: qk_chunk(1, 2, 0), lambda: qk_chunk(1, 2, 1),
                lambda: qk_chunk(0, 3, 0), lambda: qk_chunk(0, 3, 1),
            ],
            (1, 1): [
                lambda: qk_chunk(1, 3, 0), lambda: qk_chunk(1, 3, 1),
            ],
        }

        # ---- attention ----

        def emit_exp(et, ps):
            """expS^T for one head-pair tile: ACT exp on the first
            QB-SCHR_Q q-columns of each 512 block, DVE dual-Schraudolph on
            the rest (rows stay implementation-pure across kt)."""
            qs = QB - SCHR_Q
            et3 = et.rearrange("p (h q) -> p h q", h=2)
            ps3 = ps.rearrange("p (h q) -> p h q", h=2)
            if qs > 0:
                nc.scalar.activation(et3[:, :, :qs], ps3[:, :, :qs], AF.Exp)
            if SCHR_Q > 0:
                s = ps3[:, :, qs:]
                w = SCHR_Q
                y1 = rp.tile([P, 2, w], i16, tag="y1", name="y1")
                y2 = rp.tile([P, 2, w], i16, tag="y2", name="y2")
                nc.vector.tensor_scalar(y1, s, SCHR_K, SCHR_B1, MUL, ADD)
                nc.vector.tensor_scalar(y2, s, SCHR_K, SCHR_B2, MUL, ADD)
                # et = (y2.f16 * W2) + y1.f16
                nc.vector.scalar_tensor_tensor(
                    et3[:, :, qs:], y2.bitcast(f16), SCHR_W2, y1.bitcast(f16),
                    MUL, ADD,
                )

        def emit_a1(og0, eng=None):
            """Group-1 softmax normalization half of the combine (only needs
            og0, so it can run during the last q-block's g1 attention)."""
            eng = eng or nc.gpsimd
            r1 = rp.tile([P, 4], f32, tag="r", name="r1")
            nc.vector.reciprocal_approx_fast(r1, og0[:, :, VD])
            t1 = sqp.tile([P, 4, VD], f32, tag="t1", name="t1", bufs=4)
            for h in range(4):
                eng.tensor_scalar_mul(t1[:, h], og0[:, h, 0:VD], r1[:, h:h + 1])
            return t1

        def emit_combine_a(prev, qt, eng=None, t1=None):
            """Softmax-normalize both groups, differential combine, and the
            per-head sum-of-squares (fused via scalar_tensor_tensor accum).
            Pool-only math after two DVE reciprocals, so the DVE stream
            never waits on Pool."""
            og0l, og1l, _qb, oml, onl = prev[:5]
            eng = eng or nc.gpsimd
            if t1 is None:
                t1 = emit_a1(og0l[qt], eng)
            og1 = og1l[qt]
            r2 = rp.tile([P, 4], f32, tag="r", name="r2")
            nc.vector.reciprocal_approx_fast(r2, og1[:, :, VD + 1])
            o = sqp.tile([P, 4, VD], f32, tag="o", name="o", bufs=6)
            for h in range(4):
                eng.tensor_scalar_mul(o[:, h], og1[:, h, 0:VD], r2[:, h:h + 1])
            eng.tensor_add(o, o, t1)
            sq = sqp.tile([P, 4, VD], f32, tag="sq", name="sq")
            eng.tensor_mul(sq, o, o)
            ms = rp.tile([P, 4], f32, tag="r", name="ms")
            nc.vector.tensor_reduce(ms, sq, mybir.AxisListType.X, ADD)
            oml.append((o, ms))

        def emit_combine_b_ln(prev, qt):
            oml = prev[3]
            o, ms = oml[qt]
            lt = rp.tile([P, 4], f32, tag="r", name="lt")
            nc.scalar.activation(lt, ms, AF.Ln, bias=eps_sb, scale=1.0 / VD)
            oml[qt] = (o, ms, lt)

        def emit_combine_b(prev, qt, eng=None):
            """rstd = exp(-0.5*ln(ms/64+eps)); the Ln batch ran earlier so
            Ln->Exp table swaps happen once per block, not once per q-tile."""
            eng = eng or nc.gpsimd
            oml, onl = prev[3], prev[4]
            o = oml[qt][0]
            lt = oml[qt][2]
            rstd = rp.tile([P, 4], f32, tag="r", name="rstd")
            nc.scalar.activation(rstd, lt, AF.Exp, scale=-0.5)
            on = onp.tile([P, 4, VD], f16, tag="on", name="on")
            for h in range(4):
                eng.tensor_scalar_mul(on[:, h], o[:, h], rstd[:, h:h + 1])
            onl.append(on)

        def emit_transpose(prev, qt):
            onl, otl = prev[4], prev[5]
            pt = ps_s.tile([P, 2, P], f16, tag="s", name="pt")
            on2 = onl[qt].rearrange("p h v -> p (h v)")
            for c in range(2):
                nc.tensor.transpose(pt[:, c], on2[:, c * P:(c + 1) * P], ident_sb)
            ot = otp.tile([P, 2, P], f16, tag="ot", name="ot")
            nc.vector.tensor_copy(ot, pt)
            otl.append(ot)

        def emit_proj(prev, qt):
            pqb, otl = prev[2], prev[5]
            ot = otl[qt]
            pp = ps_s.tile([P, DIM], f32, tag="s", name="pp")
            for nck in range(2):
                for c in range(2):
                    nc.tensor.matmul(
                        pp[:, nck * QB:(nck + 1) * QB],
                        lhsT=ot[:, c],
                        rhs=wp_sb[:, c, nck * QB:(nck + 1) * QB],
                        start=(c == 0),
                        stop=(c == 1),
                        skip_group_check=True,
                    )
            st = stage.tile([P, DIM], f32, tag="st", name="st")
            nc.vector.tensor_copy(st, pp)
            r0 = pqb * QB + qt * P
            nc.sync.dma_start(out_d[r0:r0 + P, :], st)

        prev = None
        pre_t1 = []
        for qb in range(NQB):
            ogs = []
            for g in range(2):
                po = [
                    ps_av.tile([P, QB], f32, tag="av", name=f"po{qt}")
                    for qt in range(NQT)
                ]

                def emit_av(et, kt, h):
                    for jj in range(2):
                        j = 2 * h + jj
                        for qt in range(NQT):
                            nc.tensor.matmul(
                                po[qt][:, j * VD2:(j + 1) * VD2],
                                lhsT=et[:, jj * QB + qt * P:jj * QB + (qt + 1) * P],
                                rhs=vx_sb[kt][:, j, :],
                                start=(kt == 0 and j == 0),
                                stop=(kt == NKT - 1 and j == 3),
                                skip_group_check=True,
                            )

                pend = []
                thunks = inj.get((qb, g), [])
                for kt in range(NKT):
                    if thunks:
                        thunks.pop(0)()
                    if qb == NQB - 1 and g == 1 and kt in (2, 3, 4, 5):
                        pre_t1.append(emit_a1(ogs[0][kt - 2]))
                    if g == 0 and prev is not None:
                        if kt in (1, 2, 3, 4):
                            emit_combine_a(prev, kt - 1)
                        elif kt == 6:
                            for qt in range(NQT):
                                emit_combine_b_ln(prev, qt)
                        elif kt == 7:
                            for qt in range(NQT):
                                emit_combine_b(prev, qt)
                        elif kt == 8:
                            for qt in range(NQT):
                                emit_transpose(prev, qt)
                        elif kt in (9, 10, 11, 12):
                            emit_proj(prev, kt - 9)
                        elif kt == 13:
                            prev = None
                    for h in range(2):
                        ps = ps_s.tile([P, 2 * QB], f32, tag="s", name="ps")
                        for jj in range(2):
                            j = 2 * h + jj
                            nc.tensor.matmul(
                                ps[:, jj * QB:(jj + 1) * QB],
                                lhsT=qk_sb[2 + g][kt // NQB][
                                    HD * j:HD * (j + 1),
                                    (kt % NQB) * P:(kt % NQB + 1) * P,
                                ],
                                rhs=qk_sb[g][qb][HD * j:HD * (j + 1), :],
                                start=True,
                                stop=True,
                                tile_position=(HD * j, 0),
                            )
                        et = etp.tile([P, 2 * QB], f16, tag="e", name="et")
                        emit_exp(et, ps)
                        pend.append((et, kt, h))
                    # software pipeline: AV for tile kt runs after the
                    # scores for kt+1 are on the PE queue, so ACT always has
                    # its next input ready and the PE never gates it.
                    while len(pend) > 2:
                        emit_av(*pend.pop(0))
                for item in pend:
                    emit_av(*item)
                ogl = []
                for qt in range(NQT):
                    og = ogp.tile([P, 4, VD2], f32, tag="og", name=f"og{g}_{qt}")
                    nc.vector.tensor_copy(
                        og, po[qt][:, :4 * VD2].rearrange("p (j v) -> p j v", j=4)
                    )
                    ogl.append(og)
                ogs.append(ogl)
            prev = [ogs[0], ogs[1], qb, [], [], []]

        # tail: last q-block, per-qt pipelined across DVE/Pool/ACT/PE
        engs = [nc.gpsimd, nc.vector, nc.gpsimd, nc.vector]
        for qt in range(NQT):
            emit_combine_a(prev, qt, eng=engs[qt], t1=pre_t1[qt])
        for qt in range(NQT):
            emit_combine_b_ln(prev, qt)
        for qt in range(NQT):
            emit_combine_b(prev, qt, eng=engs[qt])
            emit_transpose(prev, qt)
            emit_proj(prev, qt)

    nc.compile()
    return nc


def _get_module():
    if "nc" not in _CACHE:
        _CACHE["nc"] = _build_module()
    return _CACHE["nc"]


def make_in_maps(inputs: dict) -> list:
    x = np.asarray(inputs["x"], np.float32)
    wqkv = np.asarray(inputs["W_qkv"], np.float32)
    wproj = np.asarray(inputs["W_proj"], np.float32)
    lq1 = np.asarray(inputs["lambda_q1"], np.float32)
    lk1 = np.asarray(inputs["lambda_k1"], np.float32)
    lq2 = np.asarray(inputs["lambda_q2"], np.float32)
    lk2 = np.asarray(inputs["lambda_k2"], np.float32)
    subw = np.asarray(inputs["subln_w"], np.float32)

    lam = float(
        np.exp(np.sum(lq1 * lk1)) - np.exp(np.sum(lq2 * lk2)) + LAMBDA_INIT
    )
    vcols = np.empty((P, NKT * 8), np.float16)
    vcols[:, 0::2] = np.float16(1.0)
    vcols[:, 1::2] = np.float16(-1.0 / lam)
    ident = np.eye(P, dtype=np.float16)
    wp_rowscale = (np.tile(subw, 4) * (1.0 - LAMBDA_INIT)).astype(np.float32)

    in_maps = []
    for c in range(NCORES):
        b, g = divmod(c, 4)
        xT = np.ascontiguousarray(x[b].T).astype(np.float16)
        ws = np.ascontiguousarray(
            np.concatenate(
                [
                    wqkv[:, 128 * g:128 * g + 128] * SCALE,
                    wqkv[:, 512 + 128 * g:512 + 128 * g + 128] * SCALE,
                    wqkv[:, 1024 + 128 * g:1024 + 128 * g + 128],
                    wqkv[:, 1536 + 128 * g:1536 + 128 * g + 128],
                    wqkv[:, 2048 + 256 * g:2048 + 256 * g + 256],
                ],
                axis=1,
            )
        ).astype(np.float16)
        wp = (wproj[256 * g:256 * (g + 1), :] * wp_rowscale[:, None]).astype(
            np.float16
        )
        # [256, 1024] -> [128, 2, 1024]: chunk c covers vd rows c*128..c*128+128
        wpd = np.ascontiguousarray(
            wp.reshape(2, P, DIM).transpose(1, 0, 2).reshape(P, 2 * DIM)
        )
        in_maps.append(
            {"xt": xT, "wqkv": ws, "wproj": wpd, "vcols": vcols, "ident": ident}
        )
    return in_maps


def combine_outputs(inputs: dict, parts: list) -> np.ndarray:
    bproj = np.asarray(inputs["b_proj"], np.float32)
    out = np.stack(
        [
            parts[0] + parts[1] + parts[2] + parts[3],
            parts[4] + parts[5] + parts[6] + parts[7],
        ]
    )
    return (out + bproj[None, None, :]).astype(np.float32)


def kernel(**inputs) -> np.ndarray:
    from concourse import bass_utils

    nc = _get_module()
    in_maps = make_in_maps(inputs)
    res = bass_utils.run_bass_kernel_spmd(nc, in_maps, core_ids=list(range(NCORES)))
    parts = [np.asarray(res.results[c]["outp"], np.float32) for c in range(NCORES)]
    return combine_outputs(inputs, parts)


# revision 41
# speedup vs baseline: 1.2209x; 1.0209x over previous
"""DiffAttention Trainium2 kernel (8-core SPMD, full-I/O contract), v2.

Sharding: core c = (batch b = c//4) x (head-group g = c%4, 4 of 16 v-heads).

Key design points (cost-model-driven rewrite of the v1 baseline):
  - All PE inputs are fp16 (1 cycle/row at ANY moving size, vs f32r which
    needs >=256).  End-to-end rel err ~6e-4 (measured in numpy), budget 2e-2.
  - Scores are computed transposed (S^T [ktok 128, qtok 512]) as before, but
    the AV matmul is flipped to out[q, vd]: lhsT = expS^T chunk [128k, 128q],
    rhs = v-head [128k, 66] -> ap=66 per matmul instead of 512.  This cuts
    the AV stream time ~7.8x (cost = moving dim only).
  - v tiles carry TWO extra columns: ones (group-1 softmax denominator) and
    -1/lambda (group-2), so both AV accumulators come out of PSUM with their
    reciprocal-ready denominators at cols 64/65 of each 66-block.
  - Softmax normalization, differential combine and RMSNorm all happen in
    the natural [q, vd] orientation: per-partition-scalar ops on Pool/DVE,
    no partition broadcasts, no GPSIMD reductions.
  - o_n is transposed back with two PE transpose ops per q-tile (ap=128) and
    projected with contract=128 (2 passes instead of 4): proj drops 2x.
  - The exp softmax (the ACT bottleneck, 256 x [128,1024] tiles) can be
    split by q-columns between ACT (table exp) and DVE (dual-Schraudolph
    bit-trick exp, 3 ops/tile-slice, rel rms ~0.5%).  Rows stay pure so the
    systematic part cancels in the softmax normalization.  SCHR_Q controls
    the number of q-columns (per 512-block) done on DVE.
  - combine/proj of q-block N are software-pipelined into q-block N+1's
    group-0 attention stream (emission order = engine order).
"""

import numpy as np

P = 128
N_TOK = 2048
DIM = 1024
NCORES = 8
HD = 32
VD = 64            # v-head dim
VD2 = VD + 2       # + ones column (g0 denom) + (-1/lam) column (g1 denom)
NKD = DIM // P     # 8 k-chunks over the model dim
NKT = N_TOK // P   # 16 token tiles
QB = 512           # query block
NQB = N_TOK // QB  # 4
NQT = QB // P      # 4 q-tiles per block
LAMBDA_INIT = 0.8 - 0.6 * float(np.exp(-0.3 * 12))
EPS = 1e-5
SCALE = HD ** -0.5

# exp split: number of q-columns (of each 512-q block) computed on DVE via
# dual-Schraudolph.  0 = everything on ACT.
SCHR_Q = 0
# dual-Schraudolph constants (fp16 bit trick, see docstring)
SCHR_K = float(1.4426950408889634 * 1024.0)
SCHR_C = 0.03
SCHR_B1 = float(15360.0 + 0.5 - SCHR_C * 1024.0 - 1024.0)
SCHR_B2 = float(15360.0 + 0.5 - SCHR_C * 1024.0 - 512.0)
SCHR_W2 = float(np.sqrt(2.0) / 2.0)

_CACHE: dict = {}


def _build_module():
    from contextlib import ExitStack

    import concourse.bass as bass  # noqa: F401
    import concourse.mybir as mybir
    import concourse.tile as tile
    from concourse import bacc

    f32 = mybir.dt.float32
    f16 = mybir.dt.float16
    i16 = mybir.dt.int16
    AF = mybir.ActivationFunctionType
    MUL = mybir.AluOpType.mult
    ADD = mybir.AluOpType.add

    nc = bacc.Bacc(
        "TRN2", target_bir_lowering=False, debug=False, num_devices=NCORES
    )

    # Pin the one ACT table set that contains every function we use
    # (exp, ln, copy, square).  Without this the table-load pass ping-pongs
    # between "exp_and_others" and "natural_log" on every RMSNorm (1.3us +
    # pipeline stall per swap).
    from concourse.hw_specs import get_activation_tables

    _tables = get_activation_tables(nc.m.arch)
    _need = {AF.Exp, AF.Ln, AF.Copy, AF.Square, AF.Identity}
    _set_id = next(
        i for i, (_n, funcs) in enumerate(_tables.items()) if _need <= funcs
    )

    xT_d = nc.dram_tensor("xt", [DIM, N_TOK], f16, kind="ExternalInput").ap()
    wqkv_d = nc.dram_tensor("wqkv", [DIM, 768], f16, kind="ExternalInput").ap()
    wproj_d = nc.dram_tensor("wproj", [P, 2 * DIM], f16, kind="ExternalInput").ap()
    vcols_d = nc.dram_tensor("vcols", [P, NKT * 8], f16, kind="ExternalInput").ap()
    ident_d = nc.dram_tensor("ident", [P, P], f16, kind="ExternalInput").ap()
    out_d = nc.dram_tensor("outp", [N_TOK, DIM], f32, kind="ExternalOutput").ap()

    with ExitStack() as ctx:
        tc = ctx.enter_context(tile.TileContext(nc))

        singles = ctx.enter_context(tc.tile_pool(name="singles", bufs=1))
        ps_s = ctx.enter_context(tc.tile_pool(name="ps_s", bufs=2, space="PSUM"))
        ps_av = ctx.enter_context(tc.tile_pool(name="ps_av", bufs=4, space="PSUM"))
        etp = ctx.enter_context(tc.tile_pool(name="etp", bufs=8))
        ogp = ctx.enter_context(tc.tile_pool(name="ogp", bufs=8))
        onp = ctx.enter_context(tc.tile_pool(name="onp", bufs=4))
        rp = ctx.enter_context(tc.tile_pool(name="rp", bufs=24))
        sqp = ctx.enter_context(tc.tile_pool(name="sqp", bufs=2))
        otp = ctx.enter_context(tc.tile_pool(name="otp", bufs=4))
        stage = ctx.enter_context(tc.tile_pool(name="stage", bufs=3))

        # qk tiles: [m][n] -> (x @ Wm)^T chunk, m in (q1, q2, k1, k2), n = tok/512
        qk_sb = [
            [singles.tile([P, QB], f16, tag=f"qk{m}_{n}", name=f"qk{m}_{n}") for n in range(NQB)]
            for m in range(4)
        ]
        vx_sb = [singles.tile([P, 4, VD2], f16, tag=f"vx{t}", name=f"vx{t}") for t in range(NKT)]
        wp_sb = singles.tile([P, 2, DIM], f16, tag="wp")
        ident_sb = singles.tile([P, P], f16, tag="id")
        eps_sb = singles.tile([P, 1], f32, tag="eps")

        nc.vector.memset(eps_sb, EPS)
        nc.scalar.add_instruction(
            mybir.InstLoadActFuncSet(
                name="act_table_pin", ins=[], outs=[], act_func_set_id=_set_id
            )
        )
        nc.sync.dma_start(ident_sb, ident_d)
        nc.sync.dma_start(wp_sb, wproj_d.rearrange("p (c n) -> p c n", c=2))

        # ---- stage 1: qkv projections ----
        # Only the prefix needed by (qb0, g0) runs before attention starts:
        # q1[n0], all k1, all v.  The remaining 11 qk groups are injected
        # into qb0's attention stream (one per odd kt) where the PE has
        # slack under the ACT-bound exp cadence.
        ph1 = ctx.enter_context(tc.tile_pool(name="ph1", bufs=1))
        xT_t = xT_d.rearrange("(ko p) t -> ko p t", p=P)
        wq_t = wqkv_d.rearrange("(ko p) c -> ko p c", p=P)
        x_sb = []
        w_sb = []
        for k in range(NKD):
            wt = ph1.tile([P, 768], f16, tag=f"w{k}", name=f"w{k}")
            nc.sync.dma_start(wt, wq_t[k])
            w_sb.append(wt)
            xt = ph1.tile([P, N_TOK], f16, tag=f"x{k}", name=f"x{k}")
            # token-block n0 lands with the weights so the first qk
            # groups start ~8us earlier; later blocks chase.
            nc.sync.dma_start(xt[:, :QB], xT_t[k][:, :QB])
            x_sb.append(xt)
        for n in range(1, NQB):
            for k in range(NKD):
                nc.sync.dma_start(
                    x_sb[k][:, n * QB:(n + 1) * QB],
                    xT_t[k][:, n * QB:(n + 1) * QB],
                )
        for t in range(NKT):
            nc.sync.dma_start(
                vx_sb[t][:, :, VD:VD2],
                vcols_d.rearrange("p (t j c) -> p t j c", j=4, c=2)[:, t],
            )

        def qk_group(m, n, eng, pool, tag):
            pqk = pool.tile([P, QB], f32, tag=tag, name="s1qk")
            for k in range(NKD):
                nc.tensor.matmul(
                    pqk,
                    lhsT=w_sb[k][:, m * P:(m + 1) * P],
                    rhs=x_sb[k][:, n * QB:(n + 1) * QB],
                    start=(k == 0),
                    stop=(k == NKD - 1),
                )
            if eng == 0:
                nc.vector.tensor_copy(qk_sb[m][n], pqk)
            else:
                nc.scalar.copy(qk_sb[m][n], pqk)

        def qk_chunk(m, n, half):
            """Half of a qk group (4 of 8 contraction chunks) as a
            self-contained transient: fits the per-kt PE slack under the
            exp cadence.  half 0 copies, half 1 accumulates via DVE add."""
            pqk = ps_s.tile([P, QB], f32, tag="s", name="s1qkc")
            for kk in range(4):
                k = 4 * half + kk
                nc.tensor.matmul(
                    pqk,
                    lhsT=w_sb[k][:, m * P:(m + 1) * P],
                    rhs=x_sb[k][:, n * QB:(n + 1) * QB],
                    start=(kk == 0),
                    stop=(kk == 3),
                )
            if half == 0:
                nc.vector.tensor_copy(qk_sb[m][n], pqk)
            else:
                nc.vector.tensor_add(qk_sb[m][n], pqk, qk_sb[m][n])

        def v_chunk(i):
            v_group(i, 0, pool=ps_s, tag="s")

        def v_group(i, eng, pool=None, tag="av"):
            pool = pool or ps_av
            pv = pool.tile([P, QB], f32, tag=tag, name="s1v")[:, :4 * VD]
            for k in range(NKD):
                nc.tensor.matmul(
                    pv,
                    lhsT=x_sb[k][:, i * P:(i + 1) * P],
                    rhs=w_sb[k][:, 512:768],
                    start=(k == 0),
                    stop=(k == NKD - 1),
                )
            dst = vx_sb[i][:, :, 0:VD]
            src = pv.rearrange("p (j v) -> p j v", j=4)
            if eng == 0:
                nc.vector.tensor_copy(dst, src)
            else:
                nc.scalar.copy(dst, src)

        INJECT = True
        # minimal prefix for (qb0, g0, kt0..7): q1[n0], k1[n0], k1[n1], v0-7
        qk_group(0, 0, 0, ps_av, "av")
        qk_group(2, 0, 1, ps_av, "av")
        for i in range(4):
            v_group(i, i % 2)
        qk_group(2, 1, 1, ps_av, "av")
        for i in range(4, 8):
            v_group(i, i % 2)
        if not INJECT:
            for i in range(8, 16):
                v_group(i, i % 2)
            qk_group(2, 2, 0, ps_av, "av")
            qk_group(2, 3, 1, ps_av, "av")
            qk_group(1, 0, 0, ps_av, "av")
            for n in range(4):
                qk_group(3, n, n % 2, ps_av, "av")
            for n in range(1, NQB):
                qk_group(0, n, n % 2, ps_av, "av")
                qk_group(1, n, (n + 1) % 2, ps_av, "av")
        # everything else is injected one chunk per kt, deadline-ordered.
        # key: (qb, g) -> list of thunks popped one per kt
        inj = {} if not INJECT else {
            (0, 0): [
                lambda: qk_chunk(2, 2, 0), lambda: qk_chunk(2, 2, 1),
                lambda: v_chunk(8), lambda: v_chunk(9), lambda: v_chunk(10),
                lambda: qk_chunk(2, 3, 0), lambda: qk_chunk(2, 3, 1),
                lambda: v_chunk(11), lambda: v_chunk(12), lambda: v_chunk(13),
                lambda: v_chunk(14), lambda: v_chunk(15),
                lambda: qk_chunk(1, 0, 0), lambda: qk_chunk(1, 0, 1),
                lambda: qk_chunk(3, 0, 0), lambda: qk_chunk(3, 0, 1),
            ],
            (0, 1): [
                lambda: qk_chunk(3, 1, 0), lambda: qk_chunk(3, 1, 1),
                lambda: qk_chunk(3, 2, 0), lambda: qk_chunk(3, 2, 1),
                lambda: qk_chunk(3, 3, 0), lambda: qk_chunk(3, 3, 1),
                lambda: qk_chunk(0, 1, 0), lambda: qk_chunk(0, 1, 1),
                lambda: qk_chunk(1, 1, 0), lambda: qk_chunk(1, 1, 1),
                lambda: qk_chunk(0, 2, 0), lambda: qk_chunk(0, 2, 1),
            ],
            (1, 0): [
                lambda: qk_chunk(1, 2, 0), lambda: qk_chunk(1, 2, 1),
                lambda: qk_chunk(0, 3, 0), lambda: qk_chunk(0, 3, 1),
            ],
            (1, 1): [
                lambda: qk_chunk(1, 3, 0), lambda: qk_chunk(1, 3, 1),
            ],
        }

        # ---- attention ----

        def emit_exp(et, ps):
            """expS^T for one head-pair tile: ACT exp on the first
            QB-SCHR_Q q-columns of each 512 block, DVE dual-Schraudolph on
            the rest (rows stay implementation-pure across kt)."""
            qs = QB - SCHR_Q
            et3 = et.rearrange("p (h q) -> p h q", h=2)
            ps3 = ps.rearrange("p (h q) -> p h q", h=2)
            if qs > 0:
                nc.scalar.activation(et3[:, :, :qs], ps3[:, :, :qs], AF.Exp)
            if SCHR_Q > 0:
                s = ps3[:, :, qs:]
                w = SCHR_Q
                y1 = rp.tile([P, 2, w], i16, tag="y1", name="y1")
                y2 = rp.tile([P, 2, w], i16, tag="y2", name="y2")
                nc.vector.tensor_scalar(y1, s, SCHR_K, SCHR_B1, MUL, ADD)
                nc.vector.tensor_scalar(y2, s, SCHR_K, SCHR_B2, MUL, ADD)
                # et = (y2.f16 * W2) + y1.f16
                nc.vector.scalar_tensor_tensor(
                    et3[:, :, qs:], y2.bitcast(f16), SCHR_W2, y1.bitcast(f16),
                    MUL, ADD,
                )

        def emit_a1(og0, eng=None):
            """Group-1 softmax normalization half of the combine (only needs
            og0, so it can run during the last q-block's g1 attention)."""
            eng = eng or nc.gpsimd
            r1 = rp.tile([P, 4], f32, tag="r", name="r1")
            nc.vector.reciprocal_approx_fast(r1, og0[:, :, VD])
            t1 = sqp.tile([P, 4, VD], f32, tag="t1", name="t1", bufs=4)
            for h in range(4):
                eng.tensor_scalar_mul(t1[:, h], og0[:, h, 0:VD], r1[:, h:h + 1])
            return t1

        def emit_combine_a(prev, qt, eng=None, t1=None):
            """Softmax-normalize both groups, differential combine, and the
            per-head sum-of-squares (fused via scalar_tensor_tensor accum).
            Pool-only math after two DVE reciprocals, so the DVE stream
            never waits on Pool."""
            og0l, og1l, _qb, oml, onl = prev[:5]
            eng = eng or nc.gpsimd
            if t1 is None:
                t1 = emit_a1(og0l[qt], eng)
            og1 = og1l[qt]
            r2 = rp.tile([P, 4], f32, tag="r", name="r2")
            nc.vector.reciprocal_approx_fast(r2, og1[:, :, VD + 1])
            o = sqp.tile([P, 4, VD], f32, tag="o", name="o", bufs=6)
            for h in range(4):
                eng.tensor_scalar_mul(o[:, h], og1[:, h, 0:VD], r2[:, h:h + 1])
            eng.tensor_add(o, o, t1)
            sq = sqp.tile([P, 4, VD], f32, tag="sq", name="sq")
            eng.tensor_mul(sq, o, o)
            ms = rp.tile([P, 4], f32, tag="r", name="ms")
            nc.vector.tensor_reduce(ms, sq, mybir.AxisListType.X, ADD)
            oml.append((o, ms))

        def emit_combine_b_ln(prev, qt):
            oml = prev[3]
            o, ms = oml[qt]
            lt = rp.tile([P, 4], f32, tag="r", name="lt")
            nc.scalar.activation(lt, ms, AF.Ln, bias=eps_sb, scale=1.0 / VD)
            oml[qt] = (o, ms, lt)

        def emit_combine_b(prev, qt, eng=None):
            """rstd = exp(-0.5*ln(ms/64+eps)); the Ln batch ran earlier so
            Ln->Exp table swaps happen once per block, not once per q-tile."""
            eng = eng or nc.gpsimd
            oml, onl = prev[3], prev[4]
            o = oml[qt][0]
            lt = oml[qt][2]
            rstd = rp.tile([P, 4], f32, tag="r", name="rstd")
            nc.scalar.activation(rstd, lt, AF.Exp, scale=-0.5)
            on = onp.tile([P, 4, VD], f16, tag="on", name="on")
            for h in range(4):
                eng.tensor_scalar_mul(on[:, h], o[:, h], rstd[:, h:h + 1])
            onl.append(on)

        def emit_transpose(prev, qt):
            onl, otl = prev[4], prev[5]
            pt = ps_s.tile([P, 2, P], f16, tag="s", name="pt")
            on2 = onl[qt].rearrange("p h v -> p (h v)")
            for c in range(2):
                nc.tensor.transpose(pt[:, c], on2[:, c * P:(c + 1) * P], ident_sb)
            ot = otp.tile([P, 2, P], f16, tag="ot", name="ot")
            nc.vector.tensor_copy(ot, pt)
            otl.append(ot)

        def emit_proj(prev, qt):
            pqb, otl = prev[2], prev[5]
            ot = otl[qt]
            pp = ps_s.tile([P, DIM], f32, tag="s", name="pp")
            for nck in range(2):
                for c in range(2):
                    nc.tensor.matmul(
                        pp[:, nck * QB:(nck + 1) * QB],
                        lhsT=ot[:, c],
                        rhs=wp_sb[:, c, nck * QB:(nck + 1) * QB],
                        start=(c == 0),
                        stop=(c == 1),
                        skip_group_check=True,
                    )
            st = stage.tile([P, DIM], f32, tag="st", name="st")
            nc.vector.tensor_copy(st, pp)
            r0 = pqb * QB + qt * P
            nc.sync.dma_start(out_d[r0:r0 + P, :], st)

        prev = None
        pre_t1 = []
        for qb in range(NQB):
            ogs = []
            for g in range(2):
                po = [
                    ps_av.tile([P, QB], f32, tag="av", name=f"po{qt}")
                    for qt in range(NQT)
                ]

                def emit_av(et, kt, h):
                    for jj in range(2):
                        j = 2 * h + jj
                        for qt in range(NQT):
                            nc.tensor.matmul(
                                po[qt][:, j * VD2:(j + 1) * VD2],
                                lhsT=et[:, jj * QB + qt * P:jj * QB + (qt + 1) * P],
                                rhs=vx_sb[kt][:, j, :],
                                start=(kt == 0 and j == 0),
                                stop=(kt == NKT - 1 and j == 3),
                                skip_group_check=True,
                            )

                pend = []
                thunks = inj.get((qb, g), [])
                for kt in range(NKT):
                    if thunks:
                        thunks.pop(0)()
                    if qb == NQB - 1 and g == 1 and kt in (2, 3, 4, 5):
                        pre_t1.append(emit_a1(ogs[0][kt - 2]))
                    if g == 0 and prev is not None:
                        if kt in (1, 2, 3, 4):
                            emit_combine_a(prev, kt - 1)
                        elif kt == 6:
                            for qt in range(NQT):
                                emit_combine_b_ln(prev, qt)
                        elif kt == 7:
                            for qt in range(NQT):
                                emit_combine_b(prev, qt)
                        elif kt == 8:
                            for qt in range(NQT):
                                emit_transpose(prev, qt)
                        elif kt in (9, 10, 11, 12):
                            emit_proj(prev, kt - 9)
                        elif kt == 13:
                            prev = None
                    for h in range(2):
                        ps = ps_s.tile([P, 2 * QB], f32, tag="s", name="ps")
                        for jj in range(2):
                            j = 2 * h + jj
                            nc.tensor.matmul(
                                ps[:, jj * QB:(jj + 1) * QB],
                                lhsT=qk_sb[2 + g][kt // NQB][
                                    HD * j:HD * (j + 1),
                                    (kt % NQB) * P:(kt % NQB + 1) * P,
                                ],
                                rhs=qk_sb[g][qb][HD * j:HD * (j + 1), :],
                                start=True,
                                stop=True,
                                tile_position=(HD * j, 0),
                            )
                        et = etp.tile([P, 2 * QB], f16, tag="e", name="et")
                        emit_exp(et, ps)
                        pend.append((et, kt, h))
                    # software pipeline: AV for tile kt runs after the
                    # scores for kt+1 are on the PE queue, so ACT always has
                    # its next input ready and the PE never gates it.
                    while len(pend) > 2:
                        emit_av(*pend.pop(0))
                for item in pend:
                    emit_av(*item)
                ogl = []
                for qt in range(NQT):
                    og = ogp.tile([P, 4, VD2], f32, tag="og", name=f"og{g}_{qt}")
                    nc.vector.tensor_copy(
                        og, po[qt][:, :4 * VD2].rearrange("p (j v) -> p j v", j=4)
                    )
                    ogl.append(og)
                ogs.append(ogl)
            prev = [ogs[0], ogs[1], qb, [], [], []]

        # tail: last q-block, per-qt pipelined across DVE/Pool/ACT/PE
        engs = [nc.gpsimd, nc.vector, nc.gpsimd, nc.vector]
        for qt in range(NQT):
            emit_combine_a(prev, qt, eng=engs[qt], t1=pre_t1[qt])
        for qt in range(NQT):
            emit_combine_b_ln(prev, qt)
        for qt in range(NQT):
            emit_combine_b(prev, qt, eng=engs[qt])
            emit_transpose(prev, qt)
            emit_proj(prev, qt)

    nc.compile()
    return nc


def _get_module():
    if "nc" not in _CACHE:
        _CACHE["nc"] = _build_module()
    return _CACHE["nc"]


def make_in_maps(inputs: dict) -> list:
    x = np.asarray(inputs["x"], np.float32)
    wqkv = np.asarray(inputs["W_qkv"], np.float32)
    wproj = np.asarray(inputs["W_proj"], np.float32)
    lq1 = np.asarray(inputs["lambda_q1"], np.float32)
    lk1 = np.asarray(inputs["lambda_k1"], np.float32)
    lq2 = np.asarray(inputs["lambda_q2"], np.float32)
    lk2 = np.asarray(inputs["lambda_k2"], np.float32)
    subw = np.asarray(inputs["subln_w"], np.float32)

    lam = float(
        np.exp(np.sum(lq1 * lk1)) - np.exp(np.sum(lq2 * lk2)) + LAMBDA_INIT
    )
    vcols = np.empty((P, NKT * 8), np.float16)
    vcols[:, 0::2] = np.float16(1.0)
    vcols[:, 1::2] = np.float16(-1.0 / lam)
    ident = np.eye(P, dtype=np.float16)
    wp_rowscale = (np.tile(subw, 4) * (1.0 - LAMBDA_INIT)).astype(np.float32)

    in_maps = []
    for c in range(NCORES):
        b, g = divmod(c, 4)
        xT = np.ascontiguousarray(x[b].T).astype(np.float16)
        ws = np.ascontiguousarray(
            np.concatenate(
                [
                    wqkv[:, 128 * g:128 * g + 128] * SCALE,
                    wqkv[:, 512 + 128 * g:512 + 128 * g + 128] * SCALE,
                    wqkv[:, 1024 + 128 * g:1024 + 128 * g + 128],
                    wqkv[:, 1536 + 128 * g:1536 + 128 * g + 128],
                    wqkv[:, 2048 + 256 * g:2048 + 256 * g + 256],
                ],
                axis=1,
            )
        ).astype(np.float16)
        wp = (wproj[256 * g:256 * (g + 1), :] * wp_rowscale[:, None]).astype(
            np.float16
        )
        # [256, 1024] -> [128, 2, 1024]: chunk c covers vd rows c*128..c*128+128
        wpd = np.ascontiguousarray(
            wp.reshape(2, P, DIM).transpose(1, 0, 2).reshape(P, 2 * DIM)
        )
        in_maps.append(
            {"xt": xT, "wqkv": ws, "wproj": wpd, "vcols": vcols, "ident": ident}
        )
    return in_maps


def combine_outputs(inputs: dict, parts: list) -> np.ndarray:
    bproj = np.asarray(inputs["b_proj"], np.float32)
    out = np.stack(
        [
            parts[0] + parts[1] + parts[2] + parts[3],
            parts[4] + parts[5] + parts[6] + parts[7],
        ]
    )
    return (out + bproj[None, None, :]).astype(np.float32)


def kernel(**inputs) -> np.ndarray:
    from concourse import bass_utils

    nc = _get_module()
    in_maps = make_in_maps(inputs)
    res = bass_utils.run_bass_kernel_spmd(nc, in_maps, core_ids=list(range(NCORES)))
    parts = [np.asarray(res.results[c]["outp"], np.float32) for c in range(NCORES)]
    return combine_outputs(inputs, parts)


# revision 44
# speedup vs baseline: 1.2226x; 1.0014x over previous
"""DiffAttention Trainium2 kernel (8-core SPMD, full-I/O contract), v2.

Sharding: core c = (batch b = c//4) x (head-group g = c%4, 4 of 16 v-heads).

Key design points (cost-model-driven rewrite of the v1 baseline):
  - All PE inputs are fp16 (1 cycle/row at ANY moving size, vs f32r which
    needs >=256).  End-to-end rel err ~6e-4 (measured in numpy), budget 2e-2.
  - Scores are computed transposed (S^T [ktok 128, qtok 512]) as before, but
    the AV matmul is flipped to out[q, vd]: lhsT = expS^T chunk [128k, 128q],
    rhs = v-head [128k, 66] -> ap=66 per matmul instead of 512.  This cuts
    the AV stream time ~7.8x (cost = moving dim only).
  - v tiles carry TWO extra columns: ones (group-1 softmax denominator) and
    -1/lambda (group-2), so both AV accumulators come out of PSUM with their
    reciprocal-ready denominators at cols 64/65 of each 66-block.
  - Softmax normalization, differential combine and RMSNorm all happen in
    the natural [q, vd] orientation: per-partition-scalar ops on Pool/DVE,
    no partition broadcasts, no GPSIMD reductions.
  - o_n is transposed back with two PE transpose ops per q-tile (ap=128) and
    projected with contract=128 (2 passes instead of 4): proj drops 2x.
  - The exp softmax (the ACT bottleneck, 256 x [128,1024] tiles) can be
    split by q-columns between ACT (table exp) and DVE (dual-Schraudolph
    bit-trick exp, 3 ops/tile-slice, rel rms ~0.5%).  Rows stay pure so the
    systematic part cancels in the softmax normalization.  SCHR_Q controls
    the number of q-columns (per 512-block) done on DVE.
  - combine/proj of q-block N are software-pipelined into q-block N+1's
    group-0 attention stream (emission order = engine order).
"""

import numpy as np

P = 128
N_TOK = 2048
DIM = 1024
NCORES = 8
HD = 32
VD = 64            # v-head dim
VD2 = VD + 2       # + ones column (g0 denom) + (-1/lam) column (g1 denom)
NKD = DIM // P     # 8 k-chunks over the model dim
NKT = N_TOK // P   # 16 token tiles
QB = 512           # query block
NQB = N_TOK // QB  # 4
NQT = QB // P      # 4 q-tiles per block
LAMBDA_INIT = 0.8 - 0.6 * float(np.exp(-0.3 * 12))
EPS = 1e-5
SCALE = HD ** -0.5

# exp split: number of q-columns (of each 512-q block) computed on DVE via
# dual-Schraudolph.  0 = everything on ACT.
SCHR_Q = 0
# dual-Schraudolph constants (fp16 bit trick, see docstring)
SCHR_K = float(1.4426950408889634 * 1024.0)
SCHR_C = 0.03
SCHR_B1 = float(15360.0 + 0.5 - SCHR_C * 1024.0 - 1024.0)
SCHR_B2 = float(15360.0 + 0.5 - SCHR_C * 1024.0 - 512.0)
SCHR_W2 = float(np.sqrt(2.0) / 2.0)

_CACHE: dict = {}


def _build_module():
    from contextlib import ExitStack

    import concourse.bass as bass  # noqa: F401
    import concourse.mybir as mybir
    import concourse.tile as tile
    from concourse import bacc

    f32 = mybir.dt.float32
    f16 = mybir.dt.float16
    i16 = mybir.dt.int16
    AF = mybir.ActivationFunctionType
    MUL = mybir.AluOpType.mult
    ADD = mybir.AluOpType.add

    nc = bacc.Bacc(
        "TRN2", target_bir_lowering=False, debug=False, num_devices=NCORES
    )

    # Pin the one ACT table set that contains every function we use
    # (exp, ln, copy, square).  Without this the table-load pass ping-pongs
    # between "exp_and_others" and "natural_log" on every RMSNorm (1.3us +
    # pipeline stall per swap).
    from concourse.hw_specs import get_activation_tables

    _tables = get_activation_tables(nc.m.arch)
    _need = {AF.Exp, AF.Ln, AF.Copy, AF.Square, AF.Identity}
    _set_id = next(
        i for i, (_n, funcs) in enumerate(_tables.items()) if _need <= funcs
    )

    xT_d = nc.dram_tensor("xt", [DIM, N_TOK], f16, kind="ExternalInput").ap()
    wqkv_d = nc.dram_tensor("wqkv", [DIM, 768], f16, kind="ExternalInput").ap()
    wproj_d = nc.dram_tensor("wproj", [P, 2 * DIM], f16, kind="ExternalInput").ap()
    vcols_d = nc.dram_tensor("vcols", [P, NKT * 8], f16, kind="ExternalInput").ap()
    ident_d = nc.dram_tensor("ident", [P, P], f16, kind="ExternalInput").ap()
    out_d = nc.dram_tensor("outp", [N_TOK, DIM], f32, kind="ExternalOutput").ap()

    with ExitStack() as ctx:
        tc = ctx.enter_context(tile.TileContext(nc))

        singles = ctx.enter_context(tc.tile_pool(name="singles", bufs=1))
        ps_s = ctx.enter_context(tc.tile_pool(name="ps_s", bufs=2, space="PSUM"))
        ps_av = ctx.enter_context(tc.tile_pool(name="ps_av", bufs=4, space="PSUM"))
        etp = ctx.enter_context(tc.tile_pool(name="etp", bufs=8))
        ogp = ctx.enter_context(tc.tile_pool(name="ogp", bufs=8))
        onp = ctx.enter_context(tc.tile_pool(name="onp", bufs=4))
        rp = ctx.enter_context(tc.tile_pool(name="rp", bufs=24))
        sqp = ctx.enter_context(tc.tile_pool(name="sqp", bufs=2))
        otp = ctx.enter_context(tc.tile_pool(name="otp", bufs=4))
        stage = ctx.enter_context(tc.tile_pool(name="stage", bufs=3))

        # qk tiles: [m][n] -> (x @ Wm)^T chunk, m in (q1, q2, k1, k2), n = tok/512
        qk_sb = [
            [singles.tile([P, QB], f16, tag=f"qk{m}_{n}", name=f"qk{m}_{n}") for n in range(NQB)]
            for m in range(4)
        ]
        vx_sb = [singles.tile([P, 4, VD2], f16, tag=f"vx{t}", name=f"vx{t}") for t in range(NKT)]
        wp_sb = singles.tile([P, 2, DIM], f16, tag="wp")
        ident_sb = singles.tile([P, P], f16, tag="id")
        eps_sb = singles.tile([P, 1], f32, tag="eps")

        nc.vector.memset(eps_sb, EPS)
        nc.scalar.add_instruction(
            mybir.InstLoadActFuncSet(
                name="act_table_pin", ins=[], outs=[], act_func_set_id=_set_id
            )
        )
        nc.sync.dma_start(ident_sb, ident_d)
        nc.sync.dma_start(wp_sb, wproj_d.rearrange("p (c n) -> p c n", c=2))

        # ---- stage 1: qkv projections ----
        # Only the prefix needed by (qb0, g0) runs before attention starts:
        # q1[n0], all k1, all v.  The remaining 11 qk groups are injected
        # into qb0's attention stream (one per odd kt) where the PE has
        # slack under the ACT-bound exp cadence.
        ph1 = ctx.enter_context(tc.tile_pool(name="ph1", bufs=1))
        xT_t = xT_d.rearrange("(ko p) t -> ko p t", p=P)
        wq_t = wqkv_d.rearrange("(ko p) c -> ko p c", p=P)
        x_sb = []
        w_sb = []
        for k in range(NKD):
            wt = ph1.tile([P, 768], f16, tag=f"w{k}", name=f"w{k}")
            nc.sync.dma_start(wt, wq_t[k])
            w_sb.append(wt)
            xt = ph1.tile([P, N_TOK], f16, tag=f"x{k}", name=f"x{k}")
            # token-block n0 lands with the weights so the first qk
            # groups start ~8us earlier; later blocks chase.
            nc.sync.dma_start(xt[:, :QB], xT_t[k][:, :QB])
            x_sb.append(xt)
        for n in range(1, NQB):
            for k in range(NKD):
                nc.sync.dma_start(
                    x_sb[k][:, n * QB:(n + 1) * QB],
                    xT_t[k][:, n * QB:(n + 1) * QB],
                )
        for t in range(NKT):
            nc.sync.dma_start(
                vx_sb[t][:, :, VD:VD2],
                vcols_d.rearrange("p (t j c) -> p t j c", j=4, c=2)[:, t],
            )

        def qk_group(m, n, eng, pool, tag):
            pqk = pool.tile([P, QB], f32, tag=tag, name="s1qk")
            for k in range(NKD):
                nc.tensor.matmul(
                    pqk,
                    lhsT=w_sb[k][:, m * P:(m + 1) * P],
                    rhs=x_sb[k][:, n * QB:(n + 1) * QB],
                    start=(k == 0),
                    stop=(k == NKD - 1),
                )
            if eng == 0:
                nc.vector.tensor_copy(qk_sb[m][n], pqk)
            else:
                nc.scalar.copy(qk_sb[m][n], pqk)

        def qk_chunk(m, n, half):
            """Half of a qk group (4 of 8 contraction chunks) as a
            self-contained transient: fits the per-kt PE slack under the
            exp cadence.  half 0 copies, half 1 accumulates via DVE add."""
            pqk = ps_s.tile([P, QB], f32, tag="s", name="s1qkc")
            for kk in range(4):
                k = 4 * half + kk
                nc.tensor.matmul(
                    pqk,
                    lhsT=w_sb[k][:, m * P:(m + 1) * P],
                    rhs=x_sb[k][:, n * QB:(n + 1) * QB],
                    start=(kk == 0),
                    stop=(kk == 3),
                )
            if half == 0:
                nc.vector.tensor_copy(qk_sb[m][n], pqk)
            else:
                nc.vector.tensor_add(qk_sb[m][n], pqk, qk_sb[m][n])

        def v_chunk(i):
            v_group(i, 0, pool=ps_s, tag="s")

        def v_group(i, eng, pool=None, tag="av"):
            pool = pool or ps_av
            pv = pool.tile([P, QB], f32, tag=tag, name="s1v")[:, :4 * VD]
            for k in range(NKD):
                nc.tensor.matmul(
                    pv,
                    lhsT=x_sb[k][:, i * P:(i + 1) * P],
                    rhs=w_sb[k][:, 512:768],
                    start=(k == 0),
                    stop=(k == NKD - 1),
                )
            dst = vx_sb[i][:, :, 0:VD]
            src = pv.rearrange("p (j v) -> p j v", j=4)
            if eng == 0:
                nc.vector.tensor_copy(dst, src)
            else:
                nc.scalar.copy(dst, src)

        INJECT = True
        # minimal prefix for (qb0, g0, kt0..7): q1[n0], k1[n0], k1[n1], v0-7
        qk_group(0, 0, 0, ps_av, "av")
        qk_group(2, 0, 1, ps_av, "av")
        for i in range(4):
            v_group(i, i % 2)
        qk_group(2, 1, 1, ps_av, "av")
        for i in range(4, 8):
            v_group(i, i % 2)
        if not INJECT:
            for i in range(8, 16):
                v_group(i, i % 2)
            qk_group(2, 2, 0, ps_av, "av")
            qk_group(2, 3, 1, ps_av, "av")
            qk_group(1, 0, 0, ps_av, "av")
            for n in range(4):
                qk_group(3, n, n % 2, ps_av, "av")
            for n in range(1, NQB):
                qk_group(0, n, n % 2, ps_av, "av")
                qk_group(1, n, (n + 1) % 2, ps_av, "av")
        # everything else is injected one chunk per kt, deadline-ordered.
        # key: (qb, g) -> list of thunks popped one per kt
        inj = {} if not INJECT else {
            (0, 0): [
                lambda: qk_chunk(2, 2, 0), lambda: qk_chunk(2, 2, 1),
                lambda: v_chunk(8), lambda: v_chunk(9), lambda: v_chunk(10),
                lambda: qk_chunk(2, 3, 0), lambda: qk_chunk(2, 3, 1),
                lambda: v_chunk(11), lambda: v_chunk(12), lambda: v_chunk(13),
                lambda: v_chunk(14), lambda: v_chunk(15),
                lambda: qk_chunk(1, 0, 0), lambda: qk_chunk(1, 0, 1),
                lambda: qk_chunk(3, 0, 0), lambda: qk_chunk(3, 0, 1),
            ],
            (0, 1): [
                lambda: qk_chunk(3, 1, 0), lambda: qk_chunk(3, 1, 1),
                lambda: qk_chunk(3, 2, 0), lambda: qk_chunk(3, 2, 1),
                lambda: qk_chunk(3, 3, 0), lambda: qk_chunk(3, 3, 1),
                lambda: qk_chunk(0, 1, 0), lambda: qk_chunk(0, 1, 1),
                lambda: qk_chunk(1, 1, 0), lambda: qk_chunk(1, 1, 1),
                lambda: qk_chunk(0, 2, 0), lambda: qk_chunk(0, 2, 1),
            ],
            (1, 0): [
                lambda: qk_chunk(1, 2, 0), lambda: qk_chunk(1, 2, 1),
                lambda: qk_chunk(0, 3, 0), lambda: qk_chunk(0, 3, 1),
            ],
            (1, 1): [
                lambda: qk_chunk(1, 3, 0), lambda: qk_chunk(1, 3, 1),
            ],
        }

        # ---- attention ----

        def emit_exp(et, ps):
            """expS^T for one head-pair tile: ACT exp on the first
            QB-SCHR_Q q-columns of each 512 block, DVE dual-Schraudolph on
            the rest (rows stay implementation-pure across kt)."""
            qs = QB - SCHR_Q
            et3 = et.rearrange("p (h q) -> p h q", h=2)
            ps3 = ps.rearrange("p (h q) -> p h q", h=2)
            if qs > 0:
                nc.scalar.activation(et3[:, :, :qs], ps3[:, :, :qs], AF.Exp)
            if SCHR_Q > 0:
                s = ps3[:, :, qs:]
                w = SCHR_Q
                y1 = rp.tile([P, 2, w], i16, tag="y1", name="y1")
                y2 = rp.tile([P, 2, w], i16, tag="y2", name="y2")
                nc.vector.tensor_scalar(y1, s, SCHR_K, SCHR_B1, MUL, ADD)
                nc.vector.tensor_scalar(y2, s, SCHR_K, SCHR_B2, MUL, ADD)
                # et = (y2.f16 * W2) + y1.f16
                nc.vector.scalar_tensor_tensor(
                    et3[:, :, qs:], y2.bitcast(f16), SCHR_W2, y1.bitcast(f16),
                    MUL, ADD,
                )

        def emit_a1(og0, eng=None):
            """Group-1 softmax normalization half of the combine (only needs
            og0, so it can run during the last q-block's g1 attention)."""
            eng = eng or nc.gpsimd
            r1 = rp.tile([P, 4], f32, tag="r", name="r1")
            nc.vector.reciprocal_approx_fast(r1, og0[:, :, VD])
            t1 = sqp.tile([P, 4, VD], f32, tag="t1", name="t1", bufs=4)
            for h in range(4):
                eng.tensor_scalar_mul(t1[:, h], og0[:, h, 0:VD], r1[:, h:h + 1])
            return t1

        def emit_combine_a(prev, qt, eng=None, t1=None):
            """Softmax-normalize both groups, differential combine, and the
            per-head sum-of-squares (fused via scalar_tensor_tensor accum).
            Pool-only math after two DVE reciprocals, so the DVE stream
            never waits on Pool."""
            og0l, og1l, _qb, oml, onl = prev[:5]
            eng = eng or nc.gpsimd
            if t1 is None:
                t1 = emit_a1(og0l[qt], eng)
            og1 = og1l[qt]
            r2 = rp.tile([P, 4], f32, tag="r", name="r2")
            nc.vector.reciprocal_approx_fast(r2, og1[:, :, VD + 1])
            o = sqp.tile([P, 4, VD], f32, tag="o", name="o", bufs=6)
            for h in range(4):
                eng.tensor_scalar_mul(o[:, h], og1[:, h, 0:VD], r2[:, h:h + 1])
            eng.tensor_add(o, o, t1)
            sq = sqp.tile([P, 4, VD], f32, tag="sq", name="sq")
            eng.tensor_mul(sq, o, o)
            ms = rp.tile([P, 4], f32, tag="r", name="ms")
            nc.vector.tensor_reduce(ms, sq, mybir.AxisListType.X, ADD)
            oml.append((o, ms))

        def emit_combine_b_ln(prev, qt):
            oml = prev[3]
            o, ms = oml[qt]
            lt = rp.tile([P, 4], f32, tag="r", name="lt")
            nc.scalar.activation(lt, ms, AF.Ln, bias=eps_sb, scale=1.0 / VD)
            oml[qt] = (o, ms, lt)

        def emit_combine_b(prev, qt, eng=None):
            """rstd = exp(-0.5*ln(ms/64+eps)); the Ln batch ran earlier so
            Ln->Exp table swaps happen once per block, not once per q-tile."""
            eng = eng or nc.gpsimd
            oml, onl = prev[3], prev[4]
            o = oml[qt][0]
            lt = oml[qt][2]
            rstd = rp.tile([P, 4], f32, tag="r", name="rstd")
            nc.scalar.activation(rstd, lt, AF.Exp, scale=-0.5)
            on = onp.tile([P, 4, VD], f16, tag="on", name="on")
            for h in range(4):
                eng.tensor_scalar_mul(on[:, h], o[:, h], rstd[:, h:h + 1])
            onl.append(on)

        def emit_transpose(prev, qt, tail=False):
            onl, otl = prev[4], prev[5]
            pt = ps_s.tile([P, 2, P], f16, tag="s", name="pt")
            on2 = onl[qt].rearrange("p h v -> p (h v)")
            for c in range(2):
                nc.tensor.transpose(pt[:, c], on2[:, c * P:(c + 1) * P], ident_sb)
            ot = otp.tile([P, 2, P], f16, tag="ot", name="ot")
            if tail:
                nc.scalar.copy(ot, pt)
            else:
                nc.vector.tensor_copy(ot, pt)
            otl.append(ot)

        def emit_proj(prev, qt, tail=False):
            pqb, otl = prev[2], prev[5]
            ot = otl[qt]
            pp = ps_s.tile([P, DIM], f32, tag="s", name="pp")
            for nck in range(2):
                for c in range(2):
                    nc.tensor.matmul(
                        pp[:, nck * QB:(nck + 1) * QB],
                        lhsT=ot[:, c],
                        rhs=wp_sb[:, c, nck * QB:(nck + 1) * QB],
                        start=(c == 0),
                        stop=(c == 1),
                        skip_group_check=True,
                    )
            st = stage.tile([P, DIM], f32, tag="st", name="st")
            if tail:
                nc.scalar.copy(st[:, :QB], pp[:, :QB])
                nc.vector.tensor_copy(st[:, QB:], pp[:, QB:])
            else:
                nc.vector.tensor_copy(st, pp)
            r0 = pqb * QB + qt * P
            nc.sync.dma_start(out_d[r0:r0 + P, :], st)

        prev = None
        pre_t1 = []
        for qb in range(NQB):
            ogs = []
            for g in range(2):
                po = [
                    ps_av.tile([P, QB], f32, tag="av", name=f"po{qt}")
                    for qt in range(NQT)
                ]

                def emit_av(et, kt, h):
                    for jj in range(2):
                        j = 2 * h + jj
                        for qt in range(NQT):
                            nc.tensor.matmul(
                                po[qt][:, j * VD2:(j + 1) * VD2],
                                lhsT=et[:, jj * QB + qt * P:jj * QB + (qt + 1) * P],
                                rhs=vx_sb[kt][:, j, :],
                                start=(kt == 0 and j == 0),
                                stop=(kt == NKT - 1 and j == 3),
                                skip_group_check=True,
                            )

                pend = []
                thunks = inj.get((qb, g), [])
                for kt in range(NKT):
                    if thunks:
                        thunks.pop(0)()
                    if qb == NQB - 1 and g == 1 and kt in (2, 3, 4, 5):
                        pre_t1.append(emit_a1(ogs[0][kt - 2]))
                    if g == 0 and prev is not None:
                        if kt in (1, 2, 3, 4):
                            emit_combine_a(prev, kt - 1)
                        elif kt == 6:
                            for qt in range(NQT):
                                emit_combine_b_ln(prev, qt)
                        elif kt == 7:
                            for qt in range(NQT):
                                emit_combine_b(prev, qt)
                        elif kt == 8:
                            for qt in range(NQT):
                                emit_transpose(prev, qt)
                        elif kt in (9, 10, 11, 12):
                            emit_proj(prev, kt - 9)
                        elif kt == 13:
                            prev = None
                    for h in range(2):
                        ps = ps_s.tile([P, 2 * QB], f32, tag="s", name="ps")
                        for jj in range(2):
                            j = 2 * h + jj
                            nc.tensor.matmul(
                                ps[:, jj * QB:(jj + 1) * QB],
                                lhsT=qk_sb[2 + g][kt // NQB][
                                    HD * j:HD * (j + 1),
                                    (kt % NQB) * P:(kt % NQB + 1) * P,
                                ],
                                rhs=qk_sb[g][qb][HD * j:HD * (j + 1), :],
                                start=True,
                                stop=True,
                                tile_position=(HD * j, 0),
                            )
                        et = etp.tile([P, 2 * QB], f16, tag="e", name="et")
                        emit_exp(et, ps)
                        pend.append((et, kt, h))
                    # software pipeline: AV for tile kt runs after the
                    # scores for kt+1 are on the PE queue, so ACT always has
                    # its next input ready and the PE never gates it.
                    while len(pend) > 2:
                        emit_av(*pend.pop(0))
                for item in pend:
                    emit_av(*item)
                ogl = []
                for qt in range(NQT):
                    og = ogp.tile([P, 4, VD2], f32, tag="og", name=f"og{g}_{qt}")
                    nc.vector.tensor_copy(
                        og, po[qt][:, :4 * VD2].rearrange("p (j v) -> p j v", j=4)
                    )
                    ogl.append(og)
                ogs.append(ogl)
            prev = [ogs[0], ogs[1], qb, [], [], []]

        # tail: last q-block, per-qt pipelined across DVE/Pool/ACT/PE
        engs = [nc.gpsimd, nc.vector, nc.gpsimd, nc.vector]
        for qt in range(NQT):
            emit_combine_a(prev, qt, eng=engs[qt], t1=pre_t1[qt])
        for qt in range(NQT):
            emit_combine_b_ln(prev, qt)
        for qt in range(NQT):
            emit_combine_b(prev, qt, eng=engs[qt])
            emit_transpose(prev, qt, tail=True)
            emit_proj(prev, qt, tail=True)

    nc.compile()
    return nc


def _get_module():
    if "nc" not in _CACHE:
        _CACHE["nc"] = _build_module()
    return _CACHE["nc"]


def make_in_maps(inputs: dict) -> list:
    x = np.asarray(inputs["x"], np.float32)
    wqkv = np.asarray(inputs["W_qkv"], np.float32)
    wproj = np.asarray(inputs["W_proj"], np.float32)
    lq1 = np.asarray(inputs["lambda_q1"], np.float32)
    lk1 = np.asarray(inputs["lambda_k1"], np.float32)
    lq2 = np.asarray(inputs["lambda_q2"], np.float32)
    lk2 = np.asarray(inputs["lambda_k2"], np.float32)
    subw = np.asarray(inputs["subln_w"], np.float32)

    lam = float(
        np.exp(np.sum(lq1 * lk1)) - np.exp(np.sum(lq2 * lk2)) + LAMBDA_INIT
    )
    vcols = np.empty((P, NKT * 8), np.float16)
    vcols[:, 0::2] = np.float16(1.0)
    vcols[:, 1::2] = np.float16(-1.0 / lam)
    ident = np.eye(P, dtype=np.float16)
    wp_rowscale = (np.tile(subw, 4) * (1.0 - LAMBDA_INIT)).astype(np.float32)

    in_maps = []
    for c in range(NCORES):
        b, g = divmod(c, 4)
        xT = np.ascontiguousarray(x[b].T).astype(np.float16)
        ws = np.ascontiguousarray(
            np.concatenate(
                [
                    wqkv[:, 128 * g:128 * g + 128] * SCALE,
                    wqkv[:, 512 + 128 * g:512 + 128 * g + 128] * SCALE,
                    wqkv[:, 1024 + 128 * g:1024 + 128 * g + 128],
                    wqkv[:, 1536 + 128 * g:1536 + 128 * g + 128],
                    wqkv[:, 2048 + 256 * g:2048 + 256 * g + 256],
                ],
                axis=1,
            )
        ).astype(np.float16)
        wp = (wproj[256 * g:256 * (g + 1), :] * wp_rowscale[:, None]).astype(
            np.float16
        )
        # [256, 1024] -> [128, 2, 1024]: chunk c covers vd rows c*128..c*128+128
        wpd = np.ascontiguousarray(
            wp.reshape(2, P, DIM).transpose(1, 0, 2).reshape(P, 2 * DIM)
        )
        in_maps.append(
            {"xt": xT, "wqkv": ws, "wproj": wpd, "vcols": vcols, "ident": ident}
        )
    return in_maps


def combine_outputs(inputs: dict, parts: list) -> np.ndarray:
    bproj = np.asarray(inputs["b_proj"], np.float32)
    out = np.stack(
        [
            parts[0] + parts[1] + parts[2] + parts[3],
            parts[4] + parts[5] + parts[6] + parts[7],
        ]
    )
    return (out + bproj[None, None, :]).astype(np.float32)


def kernel(**inputs) -> np.ndarray:
    from concourse import bass_utils

    nc = _get_module()
    in_maps = make_in_maps(inputs)
    res = bass_utils.run_bass_kernel_spmd(nc, in_maps, core_ids=list(range(NCORES)))
    parts = [np.asarray(res.results[c]["outp"], np.float32) for c in range(NCORES)]
    return combine_outputs(inputs, parts)


# revision 48
# speedup vs baseline: 1.2257x; 1.0026x over previous
"""DiffAttention Trainium2 kernel (8-core SPMD, full-I/O contract), v2.

Sharding: core c = (batch b = c//4) x (head-group g = c%4, 4 of 16 v-heads).

Key design points (cost-model-driven rewrite of the v1 baseline):
  - All PE inputs are fp16 (1 cycle/row at ANY moving size, vs f32r which
    needs >=256).  End-to-end rel err ~6e-4 (measured in numpy), budget 2e-2.
  - Scores are computed transposed (S^T [ktok 128, qtok 512]) as before, but
    the AV matmul is flipped to out[q, vd]: lhsT = expS^T chunk [128k, 128q],
    rhs = v-head [128k, 66] -> ap=66 per matmul instead of 512.  This cuts
    the AV stream time ~7.8x (cost = moving dim only).
  - v tiles carry TWO extra columns: ones (group-1 softmax denominator) and
    -1/lambda (group-2), so both AV accumulators come out of PSUM with their
    reciprocal-ready denominators at cols 64/65 of each 66-block.
  - Softmax normalization, differential combine and RMSNorm all happen in
    the natural [q, vd] orientation: per-partition-scalar ops on Pool/DVE,
    no partition broadcasts, no GPSIMD reductions.
  - o_n is transposed back with two PE transpose ops per q-tile (ap=128) and
    projected with contract=128 (2 passes instead of 4): proj drops 2x.
  - The exp softmax (the ACT bottleneck, 256 x [128,1024] tiles) can be
    split by q-columns between ACT (table exp) and DVE (dual-Schraudolph
    bit-trick exp, 3 ops/tile-slice, rel rms ~0.5%).  Rows stay pure so the
    systematic part cancels in the softmax normalization.  SCHR_Q controls
    the number of q-columns (per 512-block) done on DVE.
  - combine/proj of q-block N are software-pipelined into q-block N+1's
    group-0 attention stream (emission order = engine order).
"""

import numpy as np

P = 128
N_TOK = 2048
DIM = 1024
NCORES = 8
HD = 32
VD = 64            # v-head dim
VD2 = VD + 2       # + ones column (g0 denom) + (-1/lam) column (g1 denom)
NKD = DIM // P     # 8 k-chunks over the model dim
NKT = N_TOK // P   # 16 token tiles
QB = 512           # query block
NQB = N_TOK // QB  # 4
NQT = QB // P      # 4 q-tiles per block
LAMBDA_INIT = 0.8 - 0.6 * float(np.exp(-0.3 * 12))
EPS = 1e-5
SCALE = HD ** -0.5

# exp split: number of q-columns (of each 512-q block) computed on DVE via
# dual-Schraudolph.  0 = everything on ACT.
SCHR_Q = 0
# dual-Schraudolph constants (fp16 bit trick, see docstring)
SCHR_K = float(1.4426950408889634 * 1024.0)
SCHR_C = 0.03
SCHR_B1 = float(15360.0 + 0.5 - SCHR_C * 1024.0 - 1024.0)
SCHR_B2 = float(15360.0 + 0.5 - SCHR_C * 1024.0 - 512.0)
SCHR_W2 = float(np.sqrt(2.0) / 2.0)

_CACHE: dict = {}


def _build_module():
    from contextlib import ExitStack

    import concourse.bass as bass  # noqa: F401
    import concourse.mybir as mybir
    import concourse.tile as tile
    from concourse import bacc

    f32 = mybir.dt.float32
    f16 = mybir.dt.float16
    i16 = mybir.dt.int16
    AF = mybir.ActivationFunctionType
    MUL = mybir.AluOpType.mult
    ADD = mybir.AluOpType.add

    nc = bacc.Bacc(
        "TRN2", target_bir_lowering=False, debug=False, num_devices=NCORES
    )

    # Pin the one ACT table set that contains every function we use
    # (exp, ln, copy, square).  Without this the table-load pass ping-pongs
    # between "exp_and_others" and "natural_log" on every RMSNorm (1.3us +
    # pipeline stall per swap).
    from concourse.hw_specs import get_activation_tables

    _tables = get_activation_tables(nc.m.arch)
    _need = {AF.Exp, AF.Ln, AF.Copy, AF.Square, AF.Identity}
    _set_id = next(
        i for i, (_n, funcs) in enumerate(_tables.items()) if _need <= funcs
    )

    xT_d = nc.dram_tensor("xt", [DIM, N_TOK], f16, kind="ExternalInput").ap()
    wqkv_d = nc.dram_tensor("wqkv", [DIM, 768], f16, kind="ExternalInput").ap()
    wproj_d = nc.dram_tensor("wproj", [P, 2 * DIM], f16, kind="ExternalInput").ap()
    vcols_d = nc.dram_tensor("vcols", [P, NKT * 8], f16, kind="ExternalInput").ap()
    ident_d = nc.dram_tensor("ident", [P, P], f16, kind="ExternalInput").ap()
    out_d = nc.dram_tensor("outp", [N_TOK, DIM], f32, kind="ExternalOutput").ap()

    with ExitStack() as ctx:
        tc = ctx.enter_context(tile.TileContext(nc))

        singles = ctx.enter_context(tc.tile_pool(name="singles", bufs=1))
        ps_s = ctx.enter_context(tc.tile_pool(name="ps_s", bufs=2, space="PSUM"))
        ps_av = ctx.enter_context(tc.tile_pool(name="ps_av", bufs=4, space="PSUM"))
        etp = ctx.enter_context(tc.tile_pool(name="etp", bufs=8))
        ogp = ctx.enter_context(tc.tile_pool(name="ogp", bufs=8))
        onp = ctx.enter_context(tc.tile_pool(name="onp", bufs=4))
        rp = ctx.enter_context(tc.tile_pool(name="rp", bufs=24))
        sqp = ctx.enter_context(tc.tile_pool(name="sqp", bufs=2))
        otp = ctx.enter_context(tc.tile_pool(name="otp", bufs=4))
        stage = ctx.enter_context(tc.tile_pool(name="stage", bufs=5))

        # qk tiles: [m][n] -> (x @ Wm)^T chunk, m in (q1, q2, k1, k2), n = tok/512
        qk_sb = [
            [singles.tile([P, QB], f16, tag=f"qk{m}_{n}", name=f"qk{m}_{n}") for n in range(NQB)]
            for m in range(4)
        ]
        vx_sb = [singles.tile([P, 4, VD2], f16, tag=f"vx{t}", name=f"vx{t}") for t in range(NKT)]
        wp_sb = singles.tile([P, 2, DIM], f16, tag="wp")
        ident_sb = singles.tile([P, P], f16, tag="id")
        eps_sb = singles.tile([P, 1], f32, tag="eps")

        nc.vector.memset(eps_sb, EPS)
        nc.scalar.add_instruction(
            mybir.InstLoadActFuncSet(
                name="act_table_pin", ins=[], outs=[], act_func_set_id=_set_id
            )
        )
        nc.sync.dma_start(ident_sb, ident_d)
        nc.sync.dma_start(wp_sb, wproj_d.rearrange("p (c n) -> p c n", c=2))

        # ---- stage 1: qkv projections ----
        # Only the prefix needed by (qb0, g0) runs before attention starts:
        # q1[n0], all k1, all v.  The remaining 11 qk groups are injected
        # into qb0's attention stream (one per odd kt) where the PE has
        # slack under the ACT-bound exp cadence.
        ph1 = ctx.enter_context(tc.tile_pool(name="ph1", bufs=1))
        xT_t = xT_d.rearrange("(ko p) t -> ko p t", p=P)
        wq_t = wqkv_d.rearrange("(ko p) c -> ko p c", p=P)
        x_sb = []
        w_sb = []
        for k in range(NKD):
            wt = ph1.tile([P, 768], f16, tag=f"w{k}", name=f"w{k}")
            nc.sync.dma_start(wt, wq_t[k])
            w_sb.append(wt)
            xt = ph1.tile([P, N_TOK], f16, tag=f"x{k}", name=f"x{k}")
            # token-block n0 lands with the weights so the first qk
            # groups start ~8us earlier; later blocks chase.
            nc.sync.dma_start(xt[:, :QB], xT_t[k][:, :QB])
            x_sb.append(xt)
        for n in range(1, NQB):
            for k in range(NKD):
                nc.sync.dma_start(
                    x_sb[k][:, n * QB:(n + 1) * QB],
                    xT_t[k][:, n * QB:(n + 1) * QB],
                )
        for t in range(NKT):
            nc.sync.dma_start(
                vx_sb[t][:, :, VD:VD2],
                vcols_d.rearrange("p (t j c) -> p t j c", j=4, c=2)[:, t],
            )

        def qk_group(m, n, eng, pool, tag):
            pqk = pool.tile([P, QB], f32, tag=tag, name="s1qk")
            for k in range(NKD):
                nc.tensor.matmul(
                    pqk,
                    lhsT=w_sb[k][:, m * P:(m + 1) * P],
                    rhs=x_sb[k][:, n * QB:(n + 1) * QB],
                    start=(k == 0),
                    stop=(k == NKD - 1),
                )
            if eng == 0:
                nc.vector.tensor_copy(qk_sb[m][n], pqk)
            else:
                nc.scalar.copy(qk_sb[m][n], pqk)

        def qk_chunk(m, n, half):
            """Half of a qk group (4 of 8 contraction chunks) as a
            self-contained transient: fits the per-kt PE slack under the
            exp cadence.  half 0 copies, half 1 accumulates via DVE add."""
            pqk = ps_s.tile([P, QB], f32, tag="s", name="s1qkc")
            for kk in range(4):
                k = 4 * half + kk
                nc.tensor.matmul(
                    pqk,
                    lhsT=w_sb[k][:, m * P:(m + 1) * P],
                    rhs=x_sb[k][:, n * QB:(n + 1) * QB],
                    start=(kk == 0),
                    stop=(kk == 3),
                )
            if half == 0:
                nc.vector.tensor_copy(qk_sb[m][n], pqk)
            else:
                nc.vector.tensor_add(qk_sb[m][n], pqk, qk_sb[m][n])

        def v_chunk(i):
            v_group(i, 0, pool=ps_s, tag="s")

        def v_group(i, eng, pool=None, tag="av"):
            pool = pool or ps_av
            pv = pool.tile([P, QB], f32, tag=tag, name="s1v")[:, :4 * VD]
            for k in range(NKD):
                nc.tensor.matmul(
                    pv,
                    lhsT=x_sb[k][:, i * P:(i + 1) * P],
                    rhs=w_sb[k][:, 512:768],
                    start=(k == 0),
                    stop=(k == NKD - 1),
                )
            dst = vx_sb[i][:, :, 0:VD]
            src = pv.rearrange("p (j v) -> p j v", j=4)
            if eng == 0:
                nc.vector.tensor_copy(dst, src)
            else:
                nc.scalar.copy(dst, src)

        INJECT = True
        # minimal prefix for (qb0, g0, kt0..7): q1[n0], k1[n0], k1[n1], v0-7
        qk_group(0, 0, 0, ps_av, "av")
        qk_group(2, 0, 1, ps_av, "av")
        for i in range(4):
            v_group(i, i % 2)
        qk_group(2, 1, 1, ps_av, "av")
        for i in range(4, 8):
            v_group(i, i % 2)
        if not INJECT:
            for i in range(8, 16):
                v_group(i, i % 2)
            qk_group(2, 2, 0, ps_av, "av")
            qk_group(2, 3, 1, ps_av, "av")
            qk_group(1, 0, 0, ps_av, "av")
            for n in range(4):
                qk_group(3, n, n % 2, ps_av, "av")
            for n in range(1, NQB):
                qk_group(0, n, n % 2, ps_av, "av")
                qk_group(1, n, (n + 1) % 2, ps_av, "av")
        # everything else is injected one chunk per kt, deadline-ordered.
        # key: (qb, g) -> list of thunks popped one per kt
        inj = {} if not INJECT else {
            (0, 0): [
                lambda: qk_chunk(2, 2, 0), lambda: qk_chunk(2, 2, 1),
                lambda: v_chunk(8), lambda: v_chunk(9), lambda: v_chunk(10),
                lambda: qk_chunk(2, 3, 0), lambda: qk_chunk(2, 3, 1),
                lambda: v_chunk(11), lambda: v_chunk(12), lambda: v_chunk(13),
                lambda: v_chunk(14), lambda: v_chunk(15),
                lambda: qk_chunk(1, 0, 0), lambda: qk_chunk(1, 0, 1),
                lambda: qk_chunk(3, 0, 0), lambda: qk_chunk(3, 0, 1),
            ],
            (0, 1): [
                lambda: qk_chunk(3, 1, 0), lambda: qk_chunk(3, 1, 1),
                lambda: qk_chunk(3, 2, 0), lambda: qk_chunk(3, 2, 1),
                lambda: qk_chunk(3, 3, 0), lambda: qk_chunk(3, 3, 1),
                lambda: qk_chunk(0, 1, 0), lambda: qk_chunk(0, 1, 1),
                lambda: qk_chunk(1, 1, 0), lambda: qk_chunk(1, 1, 1),
                lambda: qk_chunk(0, 2, 0), lambda: qk_chunk(0, 2, 1),
            ],
            (1, 0): [
                lambda: qk_chunk(1, 2, 0), lambda: qk_chunk(1, 2, 1),
                lambda: qk_chunk(0, 3, 0), lambda: qk_chunk(0, 3, 1),
            ],
            (1, 1): [
                lambda: qk_chunk(1, 3, 0), lambda: qk_chunk(1, 3, 1),
            ],
        }

        # ---- attention ----

        def emit_exp(et, ps):
            """expS^T for one head-pair tile: ACT exp on the first
            QB-SCHR_Q q-columns of each 512 block, DVE dual-Schraudolph on
            the rest (rows stay implementation-pure across kt)."""
            qs = QB - SCHR_Q
            et3 = et.rearrange("p (h q) -> p h q", h=2)
            ps3 = ps.rearrange("p (h q) -> p h q", h=2)
            if qs > 0:
                nc.scalar.activation(et3[:, :, :qs], ps3[:, :, :qs], AF.Exp)
            if SCHR_Q > 0:
                s = ps3[:, :, qs:]
                w = SCHR_Q
                y1 = rp.tile([P, 2, w], i16, tag="y1", name="y1")
                y2 = rp.tile([P, 2, w], i16, tag="y2", name="y2")
                nc.vector.tensor_scalar(y1, s, SCHR_K, SCHR_B1, MUL, ADD)
                nc.vector.tensor_scalar(y2, s, SCHR_K, SCHR_B2, MUL, ADD)
                # et = (y2.f16 * W2) + y1.f16
                nc.vector.scalar_tensor_tensor(
                    et3[:, :, qs:], y2.bitcast(f16), SCHR_W2, y1.bitcast(f16),
                    MUL, ADD,
                )

        def emit_a1(og0, eng=None):
            """Group-1 softmax normalization half of the combine (only needs
            og0, so it can run during the last q-block's g1 attention)."""
            eng = eng or nc.gpsimd
            r1 = rp.tile([P, 4], f32, tag="r", name="r1")
            nc.vector.reciprocal_approx_fast(r1, og0[:, :, VD])
            t1 = sqp.tile([P, 4, VD], f32, tag="t1", name="t1", bufs=4)
            for h in range(4):
                eng.tensor_scalar_mul(t1[:, h], og0[:, h, 0:VD], r1[:, h:h + 1])
            return t1

        def emit_combine_a(prev, qt, eng=None, t1=None):
            """Softmax-normalize both groups, differential combine, and the
            per-head sum-of-squares (fused via scalar_tensor_tensor accum).
            Pool-only math after two DVE reciprocals, so the DVE stream
            never waits on Pool."""
            og0l, og1l, _qb, oml, onl = prev[:5]
            eng = eng or nc.gpsimd
            if t1 is None:
                t1 = emit_a1(og0l[qt], eng)
            og1 = og1l[qt]
            r2 = rp.tile([P, 4], f32, tag="r", name="r2")
            nc.vector.reciprocal_approx_fast(r2, og1[:, :, VD + 1])
            o = sqp.tile([P, 4, VD], f32, tag="o", name="o", bufs=6)
            for h in range(4):
                eng.tensor_scalar_mul(o[:, h], og1[:, h, 0:VD], r2[:, h:h + 1])
            eng.tensor_add(o, o, t1)
            sq = sqp.tile([P, 4, VD], f32, tag="sq", name="sq")
            eng.tensor_mul(sq, o, o)
            ms = rp.tile([P, 4], f32, tag="r", name="ms")
            nc.vector.tensor_reduce(ms, sq, mybir.AxisListType.X, ADD)
            oml.append((o, ms))

        def emit_combine_b_ln(prev, qt):
            oml = prev[3]
            o, ms = oml[qt]
            lt = rp.tile([P, 4], f32, tag="r", name="lt")
            nc.scalar.activation(lt, ms, AF.Ln, bias=eps_sb, scale=1.0 / VD)
            oml[qt] = (o, ms, lt)

        def emit_combine_b(prev, qt, eng=None):
            """rstd = exp(-0.5*ln(ms/64+eps)); the Ln batch ran earlier so
            Ln->Exp table swaps happen once per block, not once per q-tile."""
            eng = eng or nc.gpsimd
            oml, onl = prev[3], prev[4]
            o = oml[qt][0]
            lt = oml[qt][2]
            rstd = rp.tile([P, 4], f32, tag="r", name="rstd")
            nc.scalar.activation(rstd, lt, AF.Exp, scale=-0.5)
            on = onp.tile([P, 4, VD], f16, tag="on", name="on")
            for h in range(4):
                eng.tensor_scalar_mul(on[:, h], o[:, h], rstd[:, h:h + 1])
            onl.append(on)

        def emit_transpose(prev, qt, tail=False):
            onl, otl = prev[4], prev[5]
            pt = ps_s.tile([P, 2, P], f16, tag="s", name="pt")
            on2 = onl[qt].rearrange("p h v -> p (h v)")
            for c in range(2):
                nc.tensor.transpose(pt[:, c], on2[:, c * P:(c + 1) * P], ident_sb)
            ot = otp.tile([P, 2, P], f16, tag="ot", name="ot")
            if tail:
                nc.scalar.copy(ot, pt)
            else:
                nc.vector.tensor_copy(ot, pt)
            otl.append(ot)

        def emit_proj(prev, qt, tail=False):
            pqb, otl = prev[2], prev[5]
            ot = otl[qt]
            pp = ps_s.tile([P, DIM], f32, tag="s", name="pp")
            for nck in range(2):
                for c in range(2):
                    nc.tensor.matmul(
                        pp[:, nck * QB:(nck + 1) * QB],
                        lhsT=ot[:, c],
                        rhs=wp_sb[:, c, nck * QB:(nck + 1) * QB],
                        start=(c == 0),
                        stop=(c == 1),
                        skip_group_check=True,
                    )
            st = stage.tile([P, DIM], f32, tag="st", name="st")
            r0 = pqb * QB + qt * P
            if tail:
                nc.scalar.copy(st[:, :QB], pp[:, :QB])
                nc.sync.dma_start(out_d[r0:r0 + P, :QB], st[:, :QB])
                nc.vector.tensor_copy(st[:, QB:], pp[:, QB:])
                nc.sync.dma_start(out_d[r0:r0 + P, QB:], st[:, QB:])
            else:
                nc.vector.tensor_copy(st, pp)
                nc.sync.dma_start(out_d[r0:r0 + P, :], st)

        prev = None
        pre_t1 = []
        for qb in range(NQB):
            ogs = []
            for g in range(2):
                po = [
                    ps_av.tile([P, QB], f32, tag="av", name=f"po{qt}")
                    for qt in range(NQT)
                ]

                def emit_av(et, kt, h):
                    for jj in range(2):
                        j = 2 * h + jj
                        for qt in range(NQT):
                            nc.tensor.matmul(
                                po[qt][:, j * VD2:(j + 1) * VD2],
                                lhsT=et[:, jj * QB + qt * P:jj * QB + (qt + 1) * P],
                                rhs=vx_sb[kt][:, j, :],
                                start=(kt == 0 and j == 0),
                                stop=(kt == NKT - 1 and j == 3),
                                skip_group_check=True,
                            )

                pend = []
                thunks = inj.get((qb, g), [])
                for kt in range(NKT):
                    if thunks:
                        thunks.pop(0)()
                    if qb == NQB - 1 and g == 1 and kt in (2, 3, 4, 5):
                        pre_t1.append(emit_a1(ogs[0][kt - 2]))
                    if g == 0 and prev is not None:
                        if kt in (1, 2, 3, 4):
                            emit_combine_a(prev, kt - 1)
                        elif kt == 6:
                            for qt in range(NQT):
                                emit_combine_b_ln(prev, qt)
                        elif kt == 7:
                            for qt in range(NQT):
                                emit_combine_b(prev, qt)
                        elif kt == 8:
                            for qt in range(NQT):
                                emit_transpose(prev, qt)
                        elif kt in (9, 10, 11, 12):
                            emit_proj(prev, kt - 9)
                        elif kt == 13:
                            prev = None
                    for h in range(2):
                        ps = ps_s.tile([P, 2 * QB], f32, tag="s", name="ps")
                        for jj in range(2):
                            j = 2 * h + jj
                            nc.tensor.matmul(
                                ps[:, jj * QB:(jj + 1) * QB],
                                lhsT=qk_sb[2 + g][kt // NQB][
                                    HD * j:HD * (j + 1),
                                    (kt % NQB) * P:(kt % NQB + 1) * P,
                                ],
                                rhs=qk_sb[g][qb][HD * j:HD * (j + 1), :],
                                start=True,
                                stop=True,
                                tile_position=(HD * j, 0),
                            )
                        et = etp.tile([P, 2 * QB], f16, tag="e", name="et")
                        emit_exp(et, ps)
                        pend.append((et, kt, h))
                    # software pipeline: AV for tile kt runs after the
                    # scores for kt+1 are on the PE queue, so ACT always has
                    # its next input ready and the PE never gates it.
                    while len(pend) > 2:
                        emit_av(*pend.pop(0))
                for item in pend:
                    emit_av(*item)
                ogl = []
                for qt in range(NQT):
                    og = ogp.tile([P, 4, VD2], f32, tag="og", name=f"og{g}_{qt}")
                    nc.vector.tensor_copy(
                        og, po[qt][:, :4 * VD2].rearrange("p (j v) -> p j v", j=4)
                    )
                    ogl.append(og)
                ogs.append(ogl)
            prev = [ogs[0], ogs[1], qb, [], [], []]

        # tail: last q-block, per-qt pipelined across DVE/Pool/ACT/PE
        engs = [nc.gpsimd, nc.vector, nc.gpsimd, nc.vector]
        for qt in range(NQT):
            emit_combine_a(prev, qt, eng=engs[qt], t1=pre_t1[qt])
        for qt in range(NQT):
            emit_combine_b_ln(prev, qt)
        for qt in range(NQT):
            emit_combine_b(prev, qt, eng=engs[qt])
            emit_transpose(prev, qt, tail=True)
            emit_proj(prev, qt, tail=True)

    nc.compile()
    return nc


def _get_module():
    if "nc" not in _CACHE:
        _CACHE["nc"] = _build_module()
    return _CACHE["nc"]


def make_in_maps(inputs: dict) -> list:
    x = np.asarray(inputs["x"], np.float32)
    wqkv = np.asarray(inputs["W_qkv"], np.float32)
    wproj = np.asarray(inputs["W_proj"], np.float32)
    lq1 = np.asarray(inputs["lambda_q1"], np.float32)
    lk1 = np.asarray(inputs["lambda_k1"], np.float32)
    lq2 = np.asarray(inputs["lambda_q2"], np.float32)
    lk2 = np.asarray(inputs["lambda_k2"], np.float32)
    subw = np.asarray(inputs["subln_w"], np.float32)

    lam = float(
        np.exp(np.sum(lq1 * lk1)) - np.exp(np.sum(lq2 * lk2)) + LAMBDA_INIT
    )
    vcols = np.empty((P, NKT * 8), np.float16)
    vcols[:, 0::2] = np.float16(1.0)
    vcols[:, 1::2] = np.float16(-1.0 / lam)
    ident = np.eye(P, dtype=np.float16)
    wp_rowscale = (np.tile(subw, 4) * (1.0 - LAMBDA_INIT)).astype(np.float32)

    in_maps = []
    for c in range(NCORES):
        b, g = divmod(c, 4)
        xT = np.ascontiguousarray(x[b].T).astype(np.float16)
        ws = np.ascontiguousarray(
            np.concatenate(
                [
                    wqkv[:, 128 * g:128 * g + 128] * SCALE,
                    wqkv[:, 512 + 128 * g:512 + 128 * g + 128] * SCALE,
                    wqkv[:, 1024 + 128 * g:1024 + 128 * g + 128],
                    wqkv[:, 1536 + 128 * g:1536 + 128 * g + 128],
                    wqkv[:, 2048 + 256 * g:2048 + 256 * g + 256],
                ],
                axis=1,
            )
        ).astype(np.float16)
        wp = (wproj[256 * g:256 * (g + 1), :] * wp_rowscale[:, None]).astype(
            np.float16
        )
        # [256, 1024] -> [128, 2, 1024]: chunk c covers vd rows c*128..c*128+128
        wpd = np.ascontiguousarray(
            wp.reshape(2, P, DIM).transpose(1, 0, 2).reshape(P, 2 * DIM)
        )
        in_maps.append(
            {"xt": xT, "wqkv": ws, "wproj": wpd, "vcols": vcols, "ident": ident}
        )
    return in_maps


def combine_outputs(inputs: dict, parts: list) -> np.ndarray:
    bproj = np.asarray(inputs["b_proj"], np.float32)
    out = np.stack(
        [
            parts[0] + parts[1] + parts[2] + parts[3],
            parts[4] + parts[5] + parts[6] + parts[7],
        ]
    )
    return (out + bproj[None, None, :]).astype(np.float32)


def kernel(**inputs) -> np.ndarray:
    from concourse import bass_utils

    nc = _get_module()
    in_maps = make_in_maps(inputs)
    res = bass_utils.run_bass_kernel_spmd(nc, in_maps, core_ids=list(range(NCORES)))
    parts = [np.asarray(res.results[c]["outp"], np.float32) for c in range(NCORES)]
    return combine_outputs(inputs, parts)
